# revision 1
# baseline (speedup 1.0000x reference)
"""Causal attention (B=4, S=2048, D=1024, fp32) on 8 Trainium2 NeuronCores.

Sharding: data-parallel over batch (4) x query-split (2) per batch. The two
cores of a batch take interleaved query rows (even/odd within each 512-row
super-block), which makes the causal workload identical on every core and
lets one SPMD program serve all 8 cores; the only per-core differences are
pure data (which query columns of x^T each core receives, and the mask
tiles, which carry the even/odd offset).

Per core:
  qT = (x_own @ W_q)^T, kT = (x @ W_k)^T, v = x @ W_v    (fp32r matmuls)
  For each of 4 query slots s (256 queries from super-block [512s, 512s+512)):
    for key block kb in [0, 4s+4): scoresT = kT_blk^T q  -> +mask -> exp
      (no max-subtraction: scaled scores are ~N(0,1), exp is fp32-safe)
      denominators via ones-matmul; ctx accumulation in PSUM
    normalize by reciprocal(denom), DMA out.

All matmuls use fp32r (full fp32 storage, ~tf32 matmul precision, bf16-class
throughput on the PE).
"""

import numpy as np

B, S, D = 4, 2048, 1024
NE = D // 128          # contraction chunks (d on partitions)
NKBLK = S // 128       # 128-wide key blocks
NSLOT = 4              # query slots per core
QW = 256               # queries per slot
OWNQ = NSLOT * QW      # 1024 queries per core
MASK_NEG = -1.0e30
SCALE = 1.0 / 32.0     # 1/sqrt(D)

_cached = {}


def _build():
    import concourse.bacc as bacc
    import concourse.tile as tile
    import concourse.mybir as mybir

    F32 = mybir.dt.float32
    F32R = mybir.dt.float32r
    EXP = mybir.ActivationFunctionType.Exp

    nc = bacc.Bacc("TRN2", target_bir_lowering=False, debug=False, num_devices=8,
                   dynamic_dma_scratch_size=2048)

    xt_d = nc.dram_tensor("xt", [D, S], F32R, kind="ExternalInput")
    xq_d = nc.dram_tensor("xq", [D, OWNQ], F32R, kind="ExternalInput")
    wq_d = nc.dram_tensor("wq", [D, D], F32R, kind="ExternalInput")
    wk_d = nc.dram_tensor("wk", [D, D], F32R, kind="ExternalInput")
    wv_d = nc.dram_tensor("wv", [D, D], F32R, kind="ExternalInput")
    mask_d = nc.dram_tensor("masks", [128, 4 * QW], F32, kind="ExternalInput")
    ones_d = nc.dram_tensor("ones", [128, 2], F32R, kind="ExternalInput")
    o_d = nc.dram_tensor("o", [OWNQ, D], F32, kind="ExternalOutput")

    with tile.TileContext(nc) as tc:
        with tc.tile_pool(name="res", bufs=1) as res:
            kT = []
            for c in range(NE):
                t = res.tile([128, S], F32R, name=f"kT{c}", tag=f"kT{c}")
                kT.append(t)
            vv = []
            for j in range(NKBLK):
                t = res.tile([128, D], F32R, name=f"v{j}", tag=f"v{j}")
                vv.append(t)
            qT = []
            for c in range(NE):
                t = res.tile([128, OWNQ], F32R, name=f"qT{c}", tag=f"qT{c}")
                qT.append(t)
            # ---------------- projection phase ----------------
            with (
                tc.tile_pool(name="wpool", bufs=2) as wpool,
                tc.tile_pool(name="xsp", bufs=3) as xsp,
                tc.tile_pool(name="pp", bufs=6, space="PSUM") as pp,
            ):
                def load_w_half(src, col0):
                    # [d, 512]-wide half of a weight matrix, all 8 d-chunks.
                    # Chunked DMAs spread across queues; issue split over the
                    # two HWDGE engines plus idle SWDGE to cut serial issue.
                    w_t = wpool.tile([128, NE * 512], F32R, name="w_t", tag="w")
                    for dc in range(NE):
                        eng = nc.sync if dc % 2 == 0 else nc.scalar
                        eng.dma_start(
                            w_t[:, dc * 512:(dc + 1) * 512],
                            src[dc * 128:(dc + 1) * 128, col0:col0 + 512],
                        )
                    return w_t

                def load_x_slice(src, col0):
                    xs_t = xsp.tile([128, NE * QW], F32R, name="xs_t", tag="xs")
                    for dc in range(NE):
                        eng = nc.sync if dc % 2 == 0 else nc.scalar
                        eng.dma_start(
                            xs_t[:, dc * QW:(dc + 1) * QW],
                            src[dc * 128:(dc + 1) * 128, col0:col0 + QW],
                        )
                    return xs_t

                # Combined K+V pass: one stream over xt computes both
                # kT e-half h2 and v d-out half h2, halving x re-reads.
                for h2 in range(2):
                    wk_t = load_w_half(wk_d, h2 * 512)
                    wv_t = load_w_half(wv_d, h2 * 512)
                    for js in range(S // QW):
                        xs_t = load_x_slice(xt_d, js * QW)
                        for ei in range(4):
                            et = 4 * h2 + ei
                            ps = pp.tile([128, 512], F32, name="ps_p", tag="ps_p")
                            for dc in range(NE):
                                nc.tensor.matmul(
                                    ps[:, 0:QW],
                                    wk_t[:, dc * 512 + ei * 128: dc * 512 + (ei + 1) * 128],
                                    xs_t[:, dc * QW:(dc + 1) * QW],
                                    start=(dc == 0), stop=(dc == NE - 1),
                                )
                            nc.scalar.copy(kT[et][:, js * QW:(js + 1) * QW], ps[:, 0:QW])
                        for jt in range(QW // 128):
                            jc = (QW // 128) * js + jt
                            ps = pp.tile([128, 512], F32, name="ps_p", tag="ps_p")
                            for dc in range(NE):
                                nc.tensor.matmul(
                                    ps[:, 0:512],
                                    xs_t[:, dc * QW + jt * 128: dc * QW + jt * 128 + 128],
                                    wv_t[:, dc * 512:(dc + 1) * 512],
                                    start=(dc == 0), stop=(dc == NE - 1),
                                )
                            nc.vector.tensor_copy(
                                vv[jc][:, h2 * 512:(h2 + 1) * 512], ps[:, 0:512]
                            )

                # Q pass: qT[e, i] = sum_d Wq[d, e] xq[d, i]   (e-halves)
                for eh in range(2):
                    w_t = load_w_half(wq_d, eh * 512)
                    for isl in range(OWNQ // QW):
                        xs_t = load_x_slice(xq_d, isl * QW)
                        for ei in range(4):
                            et = 4 * eh + ei
                            ps = pp.tile([128, 512], F32, name="ps_p", tag="ps_p")
                            for dc in range(NE):
                                nc.tensor.matmul(
                                    ps[:, 0:QW],
                                    w_t[:, dc * 512 + ei * 128: dc * 512 + (ei + 1) * 128],
                                    xs_t[:, dc * QW:(dc + 1) * QW],
                                    start=(dc == 0), stop=(dc == NE - 1),
                                )
                            nc.scalar.copy(qT[et][:, isl * QW:(isl + 1) * QW], ps[:, 0:QW])

            # ---------------- attention phase ----------------
            with (
                tc.tile_pool(name="cns", bufs=1) as cns,
                tc.tile_pool(name="ptp", bufs=4) as ptp,
                tc.tile_pool(name="obp", bufs=2) as obp,
                tc.tile_pool(name="rcp", bufs=2) as rcp,
                tc.tile_pool(name="scp", bufs=3, space="PSUM") as scp,
                tc.tile_pool(name="ctxp", bufs=1, space="PSUM") as ctxp,
                tc.tile_pool(name="dnp", bufs=1, space="PSUM") as dnp,
            ):
                mask_t = cns.tile([128, 4 * QW], F32, name="mask_t", tag="mask_t")
                ones_t = cns.tile([128, 2], F32R, name="ones_t", tag="ones_t")
                nc.sync.dma_start(mask_t[:, :], mask_d[:, :])
                nc.sync.dma_start(ones_t[:, :], ones_d[:, :])
                def consume(item):
                    s, kb, pt, ctx, dn = item
                    nk = 4 * s + 4
                    for c in range(2):
                        # Both column groups live in one PSUM bank; start=True
                        # clears the whole bank, so only the first group may
                        # set it — the second lands on freshly cleared psum
                        # (has_written=0) and still overwrites, not adds.
                        nc.tensor.matmul(
                            dn[:, 2 * c:2 * c + 2],
                            pt[:, c * 128:(c + 1) * 128],
                            ones_t[:, :],
                            start=(kb == 0 and c == 0), stop=(kb == nk - 1),
                            skip_group_check=True,
                        )
                    for c in range(2):
                        for dh in range(2):
                            nc.tensor.matmul(
                                ctx[(c, dh)][:, :],
                                pt[:, c * 128:(c + 1) * 128],
                                vv[kb][:, dh * 512:(dh + 1) * 512],
                                start=(kb == 0), stop=(kb == nk - 1),
                            )
                    if kb == nk - 1:
                        rc = rcp.tile([128, 2], F32, name="rc", tag="rc")
                        nc.vector.reciprocal(rc[:, :], dn[:, 0:4:2])
                        for c in range(2):
                            ob = obp.tile([128, D], F32, name="ob", tag="ob")
                            for dh in range(2):
                                nc.vector.tensor_scalar_mul(
                                    ob[:, dh * 512:(dh + 1) * 512],
                                    ctx[(c, dh)][:, :],
                                    rc[:, c:c + 1],
                                )
                            nc.sync.dma_start(
                                o_d[s * QW + c * 128: s * QW + (c + 1) * 128, :],
                                ob[:, :],
                            )

                from collections import deque
                pending = deque()
                DEPTH = 2
                for s in range(NSLOT):
                    nk = 4 * s + 4
                    # Drain before each slot: the slot's ctx/dn pool slots
                    # (bufs=1) can only be re-allocated once the previous
                    # slot's normalize has been emitted.
                    while pending:
                        consume(pending.popleft())
                    ctx_cur = {}
                    for c in range(2):
                        for dh in range(2):
                            t = ctxp.tile(
                                [128, 512], F32,
                                name=f"ctx{c}{dh}", tag=f"ctx{c}{dh}",
                            )
                            ctx_cur[(c, dh)] = t
                    dn_cur = dnp.tile([128, 4], F32, name="dn", tag="dn")
                    for kb in range(nk):
                        ps_sc = scp.tile([128, QW], F32, name="ps_sc", tag="sc")
                        for ec in range(NE):
                            nc.tensor.matmul(
                                ps_sc[:, :],
                                kT[ec][:, kb * 128:(kb + 1) * 128],
                                qT[ec][:, s * QW:(s + 1) * QW],
                                start=(ec == 0), stop=(ec == NE - 1),
                            )
                        t_idx = kb - (nk - 4)
                        if t_idx >= 0:
                            nc.vector.tensor_add(
                                ps_sc[:, :], ps_sc[:, :],
                                mask_t[:, t_idx * QW:(t_idx + 1) * QW],
                            )
                        pt = ptp.tile([128, QW], F32R, name="pt", tag="pt")
                        nc.scalar.activation(pt[:, :], ps_sc[:, :], EXP, scale=SCALE)
                        pending.append((s, kb, pt, ctx_cur, dn_cur))
                        if len(pending) > DEPTH:
                            consume(pending.popleft())
                while pending:
                    consume(pending.popleft())

    nc.compile()
    return nc


def _get_nc():
    if "nc" not in _cached:
        _cached["nc"] = _build()
    return _cached["nc"]


def kernel(x, W_q, W_k, W_v):
    from concourse.bass_utils import run_bass_kernel_spmd

    x = np.asarray(x, dtype=np.float32)
    wq = np.ascontiguousarray(np.asarray(W_q, dtype=np.float32))
    wk = np.ascontiguousarray(np.asarray(W_k, dtype=np.float32))
    wv = np.ascontiguousarray(np.asarray(W_v, dtype=np.float32))
    ones = np.ones((128, 2), dtype=np.float32)

    p = np.arange(128, dtype=np.int64)[:, None]
    f = np.arange(QW, dtype=np.int64)[None, :]
    masks_h = []
    for h in range(2):
        tiles = [
            np.where(128 * t + p <= 2 * f + h, np.float32(0.0), np.float32(MASK_NEG))
            for t in range(4)
        ]
        masks_h.append(np.concatenate(tiles, axis=1).astype(np.float32))

    xt_b = [np.ascontiguousarray(x[b].T) for b in range(B)]
    in_maps = []
    for c in range(8):
        b, h = c // 2, c % 2
        xq = np.ascontiguousarray(x[b, h::2, :].T)
        in_maps.append({
            "xt": xt_b[b],
            "xq": xq,
            "wq": wq,
            "wk": wk,
            "wv": wv,
            "masks": masks_h[h],
            "ones": ones,
        })

    nc = _get_nc()
    res = run_bass_kernel_spmd(nc, in_maps, core_ids=list(range(8)))

    out = np.empty((B, S, D), dtype=np.float32)
    for c in range(8):
        b, h = c // 2, c % 2
        out[b, h::2, :] = res.results[c]["o"]
    return out



# revision 2
# speedup vs baseline: 1.1807x; 1.1807x over previous
"""Causal attention (B=4, S=2048, D=1024, fp32 in/out) on 8 Trainium2 cores.

Sharding: data-parallel over batch (4) x query-split (2) per batch. The two
cores of a batch take interleaved query rows (even/odd within each 512-row
super-block), which makes the causal workload identical on every core and
lets one SPMD program serve all 8 cores; the only per-core differences are
pure data (which query columns of x^T each core receives, and the mask
tiles, which carry the even/odd offset).

v2 changes vs the 300us baseline:
  - bf16 activations/weights (host-cast; fp32 PSUM accumulate everywhere).
    Halves HBM+SBUF traffic and drops LDWEIGHTS to ~53ns so weight loads
    hide fully under the N=512 matmul stream.
  - Projection restructured to a single pass over x with the full (bf16)
    weight matrices resident: all proj matmuls are N=512 (640 MMs instead
    of 1024 mixed N=256/512) -> per-MM overhead and LDW exposure drop.
  - Warm-up matmuls at t=0 keep the PE HAM clock-gate warm while the first
    DMAs stream, killing the 17us cold-start gap.
  - Normalize muls split across vector+scalar so slot-boundary drains don't
    queue behind the mask-add/exp chain; DEPTH=3 consume pipeline.

Per core:
  kT = (x @ W_k)^T, v = x @ W_v, qT = (x_own @ W_q)^T   (bf16 matmuls)
  For each of 4 query slots s (256 queries from super-block [512s, 512s+512)):
    for key block kb in [0, 4s+4): scoresT = kT_blk^T q -> +mask -> exp
      (no max-subtraction: scaled scores are ~N(0,1), exp is fp32-safe)
      denominators via ones-matmul; ctx accumulation in PSUM
    normalize by reciprocal(denom), DMA out (fp32).
"""

import numpy as np

B, S, D = 4, 2048, 1024
NE = D // 128          # contraction chunks (d on partitions)
NKBLK = S // 128       # 128-wide key blocks
NSLOT = 4              # query slots per core
QW = 256               # queries per slot
OWNQ = NSLOT * QW      # 1024 queries per core
MASK_NEG = -1.0e30
SCALE = 1.0 / 32.0     # 1/sqrt(D)
NWARM = 20             # PE warm-up matmuls at kernel start

_cached = {}


def _build():
    import concourse.bacc as bacc
    import concourse.tile as tile
    import concourse.mybir as mybir

    F32 = mybir.dt.float32
    BF16 = mybir.dt.bfloat16
    EXP = mybir.ActivationFunctionType.Exp

    nc = bacc.Bacc("TRN2", target_bir_lowering=False, debug=False, num_devices=8,
                   dynamic_dma_scratch_size=2048)

    xt_d = nc.dram_tensor("xt", [D, S], BF16, kind="ExternalInput")
    xq_d = nc.dram_tensor("xq", [D, OWNQ], BF16, kind="ExternalInput")
    wq_d = nc.dram_tensor("wq", [D, D], BF16, kind="ExternalInput")
    wk_d = nc.dram_tensor("wk", [D, D], BF16, kind="ExternalInput")
    wv_d = nc.dram_tensor("wv", [D, D], BF16, kind="ExternalInput")
    mask_d = nc.dram_tensor("masks", [128, 4 * QW], F32, kind="ExternalInput")
    warm_d = nc.dram_tensor("warm", [128, 640], BF16, kind="ExternalInput")
    o_d = nc.dram_tensor("o", [OWNQ, D], F32, kind="ExternalOutput")

    with tile.TileContext(nc) as tc:
        with tc.tile_pool(name="res", bufs=1) as res:
            # column layouts: kT chunk ec at [ec*S + key], v block kb at
            # [kb*D + dout], qT chunk ec at [ec*OWNQ + q]
            kT = res.tile([128, NE * S], BF16, name="kT", tag="kT")
            vv = res.tile([128, NKBLK * D], BF16, name="vv", tag="vv")
            qT = res.tile([128, NE * OWNQ], BF16, name="qT", tag="qT")
            warm_t = res.tile([128, 640], BF16, name="warm_t", tag="warm_t")
            mask_t = res.tile([128, 4 * QW], F32, name="mask_t", tag="mask_t")
            nc.sync.dma_start(warm_t[:, :], warm_d[:, :])
            nc.scalar.dma_start(mask_t[:, :], mask_d[:, :])

            # ---------------- projection phase ----------------
            with (
                tc.tile_pool(name="wpool", bufs=3) as wpool,
                tc.tile_pool(name="xpool", bufs=1) as xpool,
                tc.tile_pool(name="pp", bufs=6, space="PSUM") as pp,
                tc.tile_pool(name="wmp", bufs=2, space="PSUM") as wmp,
            ):
                # Warm-up matmuls on the (tiny, early-arriving) warm tile:
                # keeps PE activity up while the big input DMAs stream, so
                # the HAM clock-gate reaches 2.4GHz before real work starts.
                for i in range(NWARM):
                    wps = wmp.tile([128, 256], F32, name="wps", tag="wps")
                    nc.tensor.matmul(wps[:, :], warm_t[:, 0:128],
                                     warm_t[:, 128:384], start=True, stop=True)

                def load_w(dst, src):
                    # full [D, D] weight as 8 chunk DMAs, alternating queues
                    for dc in range(NE):
                        eng = nc.sync if dc % 2 == 0 else nc.scalar
                        eng.dma_start(
                            dst[:, dc * D:(dc + 1) * D],
                            src[dc * 128:(dc + 1) * 128, :],
                        )

                wk_t = wpool.tile([128, NE * D], BF16, name="wk_t", tag="w")
                xt_t = xpool.tile([128, NE * S], BF16, name="xt_t", tag="xt")
                # wk first (first K matmul group needs all of it), xt
                # interleaved in fine [128, 1024] chunks for early start.
                load_w(wk_t, wk_d)
                for dc in range(NE):
                    for h in range(2):
                        eng = nc.sync if (2 * dc + h) % 2 == 0 else nc.scalar
                        eng.dma_start(
                            xt_t[:, dc * S + h * 1024: dc * S + (h + 1) * 1024],
                            xt_d[dc * 128:(dc + 1) * 128, h * 1024:(h + 1) * 1024],
                        )
                wv_t = wpool.tile([128, NE * D], BF16, name="wv_t", tag="w")
                load_w(wv_t, wv_d)
                wq_t = wpool.tile([128, NE * D], BF16, name="wq_t", tag="w")
                load_w(wq_t, wq_d)
                xq_t = xpool.tile([128, NE * OWNQ], BF16, name="xq_t", tag="xq")
                for dc in range(NE):
                    eng = nc.sync if dc % 2 == 0 else nc.scalar
                    eng.dma_start(
                        xq_t[:, dc * OWNQ:(dc + 1) * OWNQ],
                        xq_d[dc * 128:(dc + 1) * 128, :],
                    )

                # K + V in one pass over the resident x^T; all MMs N=512.
                for js in range(S // 512):
                    for ei in range(NE):
                        ps = pp.tile([128, 512], F32, name="ps_p", tag="ps_p")
                        for dc in range(NE):
                            nc.tensor.matmul(
                                ps[:, :],
                                wk_t[:, dc * D + ei * 128: dc * D + (ei + 1) * 128],
                                xt_t[:, dc * S + js * 512: dc * S + (js + 1) * 512],
                                start=(dc == 0), stop=(dc == NE - 1),
                            )
                        nc.scalar.copy(kT[:, ei * S + js * 512: ei * S + (js + 1) * 512],
                                       ps[:, :])
                    for jt in range(4):
                        kb = 4 * js + jt
                        for dh in range(2):
                            ps = pp.tile([128, 512], F32, name="ps_p", tag="ps_p")
                            for dc in range(NE):
                                nc.tensor.matmul(
                                    ps[:, :],
                                    xt_t[:, dc * S + kb * 128: dc * S + kb * 128 + 128],
                                    wv_t[:, dc * D + dh * 512: dc * D + (dh + 1) * 512],
                                    start=(dc == 0), stop=(dc == NE - 1),
                                )
                            nc.vector.tensor_copy(
                                vv[:, kb * D + dh * 512: kb * D + (dh + 1) * 512],
                                ps[:, :],
                            )

                # Q pass: qT[e, i] = sum_d Wq[d, e] xq[d, i]
                for isl in range(OWNQ // 512):
                    for ei in range(NE):
                        ps = pp.tile([128, 512], F32, name="ps_p", tag="ps_p")
                        for dc in range(NE):
                            nc.tensor.matmul(
                                ps[:, :],
                                wq_t[:, dc * D + ei * 128: dc * D + (ei + 1) * 128],
                                xq_t[:, dc * OWNQ + isl * 512: dc * OWNQ + (isl + 1) * 512],
                                start=(dc == 0), stop=(dc == NE - 1),
                            )
                        nc.scalar.copy(
                            qT[:, ei * OWNQ + isl * 512: ei * OWNQ + (isl + 1) * 512],
                            ps[:, :])

            # ---------------- attention phase ----------------
            with (
                tc.tile_pool(name="ptp", bufs=6) as ptp,
                tc.tile_pool(name="obp", bufs=2) as obp,
                tc.tile_pool(name="rcp", bufs=2) as rcp,
                tc.tile_pool(name="scp", bufs=3, space="PSUM") as scp,
                tc.tile_pool(name="ctxp", bufs=1, space="PSUM") as ctxp,
                tc.tile_pool(name="dnp", bufs=1, space="PSUM") as dnp,
            ):
                def consume(item):
                    s, kb, pt, ctx, dn = item
                    nk = 4 * s + 4
                    for c in range(2):
                        # Both column groups live in one PSUM bank; start=True
                        # clears the whole bank, so only the first group may
                        # set it — the second lands on freshly cleared psum
                        # (has_written=0) and still overwrites, not adds.
                        nc.tensor.matmul(
                            dn[:, 2 * c:2 * c + 2],
                            pt[:, c * 128:(c + 1) * 128],
                            warm_t[:, 0:2],
                            start=(kb == 0 and c == 0), stop=(kb == nk - 1),
                            skip_group_check=True,
                        )
                    for c in range(2):
                        for dh in range(2):
                            nc.tensor.matmul(
                                ctx[(c, dh)][:, :],
                                pt[:, c * 128:(c + 1) * 128],
                                vv[:, kb * D + dh * 512: kb * D + (dh + 1) * 512],
                                start=(kb == 0), stop=(kb == nk - 1),
                            )
                    if kb == nk - 1:
                        rc = rcp.tile([128, 2], F32, name="rc", tag="rc")
                        nc.vector.reciprocal(rc[:, :], dn[:, 0:4:2])
                        for c in range(2):
                            ob = obp.tile([128, D], F32, name="ob", tag="ob")
                            for dh in range(2):
                                # split the 4 normalize muls across vector +
                                # scalar so neither queue stalls the next
                                # slot's mask-add/exp chain
                                eng = nc.vector if c == 0 else nc.scalar
                                if eng is nc.vector:
                                    eng.tensor_scalar_mul(
                                        ob[:, dh * 512:(dh + 1) * 512],
                                        ctx[(c, dh)][:, :],
                                        rc[:, c:c + 1],
                                    )
                                else:
                                    eng.mul(
                                        ob[:, dh * 512:(dh + 1) * 512],
                                        ctx[(c, dh)][:, :],
                                        rc[:, c:c + 1],
                                    )
                            eng_d = nc.sync if c == 0 else nc.scalar
                            eng_d.dma_start(
                                o_d[s * QW + c * 128: s * QW + (c + 1) * 128, :],
                                ob[:, :],
                            )

                from collections import deque
                pending = deque()
                DEPTH = 3
                for s in range(NSLOT):
                    nk = 4 * s + 4
                    # Drain before each slot: the slot's ctx/dn pool slots
                    # (bufs=1) can only be re-allocated once the previous
                    # slot's normalize has been emitted.
                    while pending:
                        consume(pending.popleft())
                    ctx_cur = {}
                    for c in range(2):
                        for dh in range(2):
                            t = ctxp.tile(
                                [128, 512], F32,
                                name=f"ctx{c}{dh}", tag=f"ctx{c}{dh}",
                            )
                            ctx_cur[(c, dh)] = t
                    dn_cur = dnp.tile([128, 4], F32, name="dn", tag="dn")
                    for kb in range(nk):
                        ps_sc = scp.tile([128, QW], F32, name="ps_sc", tag="sc")
                        for ec in range(NE):
                            nc.tensor.matmul(
                                ps_sc[:, :],
                                kT[:, ec * S + kb * 128: ec * S + (kb + 1) * 128],
                                qT[:, ec * OWNQ + s * QW: ec * OWNQ + (s + 1) * QW],
                                start=(ec == 0), stop=(ec == NE - 1),
                            )
                        t_idx = kb - (nk - 4)
                        if t_idx >= 0:
                            nc.vector.tensor_add(
                                ps_sc[:, :], ps_sc[:, :],
                                mask_t[:, t_idx * QW:(t_idx + 1) * QW],
                            )
                        pt = ptp.tile([128, QW], BF16, name="pt", tag="pt")
                        nc.scalar.activation(pt[:, :], ps_sc[:, :], EXP, scale=SCALE)
                        pending.append((s, kb, pt, ctx_cur, dn_cur))
                        if len(pending) > DEPTH:
                            consume(pending.popleft())
                while pending:
                    consume(pending.popleft())

    nc.compile()
    return nc


def _get_nc():
    if "nc" not in _cached:
        _cached["nc"] = _build()
    return _cached["nc"]


def build_in_maps(x, W_q, W_k, W_v):
    import ml_dtypes

    BF = ml_dtypes.bfloat16
    x = np.asarray(x, dtype=np.float32)
    wq = np.ascontiguousarray(np.asarray(W_q, dtype=BF))
    wk = np.ascontiguousarray(np.asarray(W_k, dtype=BF))
    wv = np.ascontiguousarray(np.asarray(W_v, dtype=BF))
    warm = np.ones((128, 640), dtype=BF)

    p = np.arange(128, dtype=np.int64)[:, None]
    f = np.arange(QW, dtype=np.int64)[None, :]
    masks_h = []
    for h in range(2):
        tiles = [
            np.where(128 * t + p <= 2 * f + h, np.float32(0.0), np.float32(MASK_NEG))
            for t in range(4)
        ]
        masks_h.append(np.concatenate(tiles, axis=1).astype(np.float32))

    xt_b = [np.ascontiguousarray(x[b].T.astype(BF)) for b in range(B)]
    in_maps = []
    for c in range(8):
        b, h = c // 2, c % 2
        xq = np.ascontiguousarray(x[b, h::2, :].T.astype(BF))
        in_maps.append({
            "xt": xt_b[b],
            "xq": xq,
            "wq": wq,
            "wk": wk,
            "wv": wv,
            "masks": masks_h[h],
            "warm": warm,
        })
    return in_maps


def kernel(x, W_q, W_k, W_v):
    from concourse.bass_utils import run_bass_kernel_spmd

    in_maps = build_in_maps(x, W_q, W_k, W_v)
    nc = _get_nc()
    res = run_bass_kernel_spmd(nc, in_maps, core_ids=list(range(8)))

    out = np.empty((B, S, D), dtype=np.float32)
    for c in range(8):
        b, h = c // 2, c % 2
        out[b, h::2, :] = res.results[c]["o"]
    return out


# revision 6
# speedup vs baseline: 1.2035x; 1.0193x over previous
"""Causal attention (B=4, S=2048, D=1024, fp32 in/out) on 8 Trainium2 cores.

Sharding: data-parallel over batch (4) x query-split (2) per batch. The two
cores of a batch take interleaved query rows (even/odd within each 512-row
super-block), which makes the causal workload identical on every core and
lets one SPMD program serve all 8 cores; the only per-core differences are
pure data (which query columns of x^T each core receives, and the mask
tiles, which carry the even/odd offset).

v2 changes vs the 300us baseline:
  - bf16 activations/weights (host-cast; fp32 PSUM accumulate everywhere).
    Halves HBM+SBUF traffic and drops LDWEIGHTS to ~53ns so weight loads
    hide fully under the N=512 matmul stream.
  - Projection restructured to a single pass over x with the full (bf16)
    weight matrices resident: all proj matmuls are N=512 (640 MMs instead
    of 1024 mixed N=256/512) -> per-MM overhead and LDW exposure drop.
  - Warm-up matmuls at t=0 keep the PE HAM clock-gate warm while the first
    DMAs stream, killing the 17us cold-start gap.
  - Normalize muls split across vector+scalar so slot-boundary drains don't
    queue behind the mask-add/exp chain; DEPTH=3 consume pipeline.

Per core:
  kT = (x @ W_k)^T, v = x @ W_v, qT = (x_own @ W_q)^T   (bf16 matmuls)
  For each of 4 query slots s (256 queries from super-block [512s, 512s+512)):
    for key block kb in [0, 4s+4): scoresT = kT_blk^T q -> +mask -> exp
      (no max-subtraction: scaled scores are ~N(0,1), exp is fp32-safe)
      denominators via ones-matmul; ctx accumulation in PSUM
    normalize by reciprocal(denom), DMA out (fp32).
"""

import numpy as np

B, S, D = 4, 2048, 1024
NE = D // 128          # contraction chunks (d on partitions)
NKBLK = S // 128       # 128-wide key blocks
NSLOT = 4              # query slots per core
QW = 256               # queries per slot
OWNQ = NSLOT * QW      # 1024 queries per core
MASK_NEG = -1.0e30
SCALE = 1.0 / 32.0     # 1/sqrt(D)
NWARM = 20             # PE warm-up matmuls at kernel start

_cached = {}


def _build():
    import concourse.bacc as bacc
    import concourse.tile as tile
    import concourse.mybir as mybir

    F32 = mybir.dt.float32
    BF16 = mybir.dt.bfloat16
    EXP = mybir.ActivationFunctionType.Exp

    nc = bacc.Bacc("TRN2", target_bir_lowering=False, debug=False, num_devices=8,
                   dynamic_dma_scratch_size=2048)

    xt_d = nc.dram_tensor("xt", [D, S], BF16, kind="ExternalInput")
    xq_d = nc.dram_tensor("xq", [D, OWNQ], BF16, kind="ExternalInput")
    wq_d = nc.dram_tensor("wq", [D, D], BF16, kind="ExternalInput")
    wk_d = nc.dram_tensor("wk", [D, D], BF16, kind="ExternalInput")
    wv_d = nc.dram_tensor("wv", [D, D], BF16, kind="ExternalInput")
    mask_d = nc.dram_tensor("masks", [128, 4 * QW], F32, kind="ExternalInput")
    warm_d = nc.dram_tensor("warm", [128, 640], BF16, kind="ExternalInput")
    o_d = nc.dram_tensor("o", [OWNQ, D], F32, kind="ExternalOutput")

    with tile.TileContext(nc) as tc:
        with tc.tile_pool(name="res", bufs=1) as res:
            # column layouts: kT chunk ec at [ec*S + key], v block kb at
            # [kb*D + dout], qT chunk ec at [ec*OWNQ + q]
            kT = res.tile([128, NE * S], BF16, name="kT", tag="kT")
            vv = res.tile([128, NKBLK * D], BF16, name="vv", tag="vv")
            qT = res.tile([128, NE * OWNQ], BF16, name="qT", tag="qT")
            warm_t = res.tile([128, 640], BF16, name="warm_t", tag="warm_t")
            mask_t = res.tile([128, 4 * QW], F32, name="mask_t", tag="mask_t")
            nc.sync.dma_start(warm_t[:, :], warm_d[:, :])
            nc.scalar.dma_start(mask_t[:, :], mask_d[:, :])

            # ---------------- projection phase ----------------
            with (
                tc.tile_pool(name="wpool", bufs=3) as wpool,
                tc.tile_pool(name="xpool", bufs=1) as xpool,
                tc.tile_pool(name="pp", bufs=6, space="PSUM") as pp,
                tc.tile_pool(name="wmp", bufs=2, space="PSUM") as wmp,
            ):
                # Warm-up matmuls on the (tiny, early-arriving) warm tile:
                # keeps PE activity up while the big input DMAs stream, so
                # the HAM clock-gate reaches 2.4GHz before real work starts.
                for i in range(NWARM):
                    wps = wmp.tile([128, 256], F32, name="wps", tag="wps")
                    nc.tensor.matmul(wps[:, :], warm_t[:, 0:128],
                                     warm_t[:, 128:384], start=True, stop=True)

                def load_w(dst, src):
                    # full [D, D] weight as 8 chunk DMAs, alternating queues
                    for dc in range(NE):
                        eng = nc.sync if dc % 2 == 0 else nc.scalar
                        eng.dma_start(
                            dst[:, dc * D:(dc + 1) * D],
                            src[dc * 128:(dc + 1) * 128, :],
                        )

                wk_t = wpool.tile([128, NE * D], BF16, name="wk_t", tag="w")
                xt_t = xpool.tile([128, NE * S], BF16, name="xt_t", tag="xt")
                # DMA order tracks first use: (wk chunk dc + xt h0 chunk dc)
                # pairs feed the first K groups ASAP, then wv (V starts after
                # one K block), then xt h1 (used from js=2), then wq, xq.
                def load_xt_half(h):
                    for dc in range(NE):
                        eng = nc.sync if dc % 2 == 0 else nc.scalar
                        eng.dma_start(
                            xt_t[:, dc * S + h * 1024: dc * S + (h + 1) * 1024],
                            xt_d[dc * 128:(dc + 1) * 128, h * 1024:(h + 1) * 1024],
                        )

                for dc in range(NE):
                    eng = nc.sync if dc % 2 == 0 else nc.scalar
                    eng.dma_start(wk_t[:, dc * D:(dc + 1) * D],
                                  wk_d[dc * 128:(dc + 1) * 128, :])
                    eng2 = nc.scalar if dc % 2 == 0 else nc.sync
                    eng2.dma_start(
                        xt_t[:, dc * S: dc * S + 1024],
                        xt_d[dc * 128:(dc + 1) * 128, 0:1024],
                    )
                wv_t = wpool.tile([128, NE * D], BF16, name="wv_t", tag="w")
                load_w(wv_t, wv_d)
                load_xt_half(1)
                wq_t = wpool.tile([128, NE * D], BF16, name="wq_t", tag="w")
                load_w(wq_t, wq_d)
                xq_t = xpool.tile([128, NE * OWNQ], BF16, name="xq_t", tag="xq")
                for dc in range(NE):
                    eng = nc.sync if dc % 2 == 0 else nc.scalar
                    eng.dma_start(
                        xq_t[:, dc * OWNQ:(dc + 1) * OWNQ],
                        xq_d[dc * 128:(dc + 1) * 128, :],
                    )

                # K + V in one pass over the resident x^T; all MMs N=512.
                for js in range(S // 512):
                    for ei in range(NE):
                        ps = pp.tile([128, 512], F32, name="ps_p", tag="ps_p")
                        for dc in range(NE):
                            nc.tensor.matmul(
                                ps[:, :],
                                wk_t[:, dc * D + ei * 128: dc * D + (ei + 1) * 128],
                                xt_t[:, dc * S + js * 512: dc * S + (js + 1) * 512],
                                start=(dc == 0), stop=(dc == NE - 1),
                            )
                        nc.scalar.copy(kT[:, ei * S + js * 512: ei * S + (js + 1) * 512],
                                       ps[:, :])
                    for jt in range(4):
                        kb = 4 * js + jt
                        for dh in range(2):
                            ps = pp.tile([128, 512], F32, name="ps_p", tag="ps_p")
                            for dc in range(NE):
                                nc.tensor.matmul(
                                    ps[:, :],
                                    xt_t[:, dc * S + kb * 128: dc * S + kb * 128 + 128],
                                    wv_t[:, dc * D + dh * 512: dc * D + (dh + 1) * 512],
                                    start=(dc == 0), stop=(dc == NE - 1),
                                )
                            nc.vector.tensor_copy(
                                vv[:, kb * D + dh * 512: kb * D + (dh + 1) * 512],
                                ps[:, :],
                            )

                # Q pass: qT[e, i] = sum_d Wq[d, e] xq[d, i]
                for isl in range(OWNQ // 512):
                    for ei in range(NE):
                        ps = pp.tile([128, 512], F32, name="ps_p", tag="ps_p")
                        for dc in range(NE):
                            nc.tensor.matmul(
                                ps[:, :],
                                wq_t[:, dc * D + ei * 128: dc * D + (ei + 1) * 128],
                                xq_t[:, dc * OWNQ + isl * 512: dc * OWNQ + (isl + 1) * 512],
                                start=(dc == 0), stop=(dc == NE - 1),
                            )
                        nc.scalar.copy(
                            qT[:, ei * OWNQ + isl * 512: ei * OWNQ + (isl + 1) * 512],
                            ps[:, :])

            # ---------------- attention phase ----------------
            with (
                tc.tile_pool(name="ptp", bufs=6) as ptp,
                tc.tile_pool(name="obp", bufs=2) as obp,
                tc.tile_pool(name="rcp", bufs=2) as rcp,
                tc.tile_pool(name="scp", bufs=3, space="PSUM") as scp,
                tc.tile_pool(name="ctxp", bufs=1, space="PSUM") as ctxp,
                tc.tile_pool(name="dnp", bufs=1, space="PSUM") as dnp,
            ):
                def consume(item):
                    s, kb, pt, ctx, dn = item
                    nk = 4 * s + 4
                    t_idx = kb - (nk - 4)
                    # Diagonal trim: for diag block t, only queries f >= 64*t
                    # can attend (the rest are fully masked). Scores/exp were
                    # computed only on [q0, 256); dn/ctx must restrict their
                    # stationary (pt) columns the same way — pt[:, :q0] is
                    # uninitialized garbage.
                    q0 = 64 * t_idx if t_idx > 0 else 0
                    for c in range(2):
                        lo = max(q0, c * 128)
                        hi = (c + 1) * 128
                        if lo >= hi:
                            continue  # whole c-half masked for this block
                        # c=0 ends early on trimmed slots (its last two diag
                        # blocks are skipped): close its accumulation group
                        # on its last executed block.
                        last_kb = (nk - 3) if c == 0 else (nk - 1)
                        # Both column groups live in one PSUM bank; start=True
                        # clears the whole bank, so only the first group may
                        # set it — the second lands on freshly cleared psum
                        # (has_written=0) and still overwrites, not adds.
                        nc.tensor.matmul(
                            dn[lo - c * 128: hi - c * 128, 2 * c:2 * c + 2],
                            pt[:, lo:hi],
                            warm_t[:, 0:2],
                            start=(kb == 0 and c == 0), stop=(kb == last_kb),
                            skip_group_check=True,
                        )
                        for dh in range(2):
                            nc.tensor.matmul(
                                ctx[(c, dh)][lo - c * 128: hi - c * 128, :],
                                pt[:, lo:hi],
                                vv[:, kb * D + dh * 512: kb * D + (dh + 1) * 512],
                                start=(kb == 0), stop=(kb == last_kb),
                                skip_group_check=True,
                            )
                    if kb == nk - 1:
                        rc = rcp.tile([128, 2], F32, name="rc", tag="rc")
                        nc.vector.reciprocal(rc[:, :], dn[:, 0:4:2])
                        for c in range(2):
                            ob = obp.tile([128, D], F32, name="ob", tag="ob")
                            for dh in range(2):
                                # split the 4 normalize muls across vector +
                                # scalar so neither queue stalls the next
                                # slot's mask-add/exp chain
                                eng = nc.vector if c == 0 else nc.scalar
                                if eng is nc.vector:
                                    eng.tensor_scalar_mul(
                                        ob[:, dh * 512:(dh + 1) * 512],
                                        ctx[(c, dh)][:, :],
                                        rc[:, c:c + 1],
                                    )
                                else:
                                    eng.mul(
                                        ob[:, dh * 512:(dh + 1) * 512],
                                        ctx[(c, dh)][:, :],
                                        rc[:, c:c + 1],
                                    )
                            eng_d = nc.sync if c == 0 else nc.scalar
                            eng_d.dma_start(
                                o_d[s * QW + c * 128: s * QW + (c + 1) * 128, :],
                                ob[:, :],
                            )

                from collections import deque
                pending = deque()
                DEPTH = 3
                for s in range(NSLOT):
                    nk = 4 * s + 4
                    # Drain before each slot: the slot's ctx/dn pool slots
                    # (bufs=1) can only be re-allocated once the previous
                    # slot's normalize has been emitted.
                    while pending:
                        consume(pending.popleft())
                    ctx_cur = {}
                    for c in range(2):
                        for dh in range(2):
                            t = ctxp.tile(
                                [128, 512], F32,
                                name=f"ctx{c}{dh}", tag=f"ctx{c}{dh}",
                            )
                            ctx_cur[(c, dh)] = t
                    dn_cur = dnp.tile([128, 4], F32, name="dn", tag="dn")
                    for kb in range(nk):
                        t_idx = kb - (nk - 4)
                        q0 = 64 * t_idx if t_idx > 0 else 0
                        qn = QW - q0
                        ps_sc = scp.tile([128, QW], F32, name="ps_sc", tag="sc")
                        for ec in range(NE):
                            nc.tensor.matmul(
                                ps_sc[:, q0:QW],
                                kT[:, ec * S + kb * 128: ec * S + (kb + 1) * 128],
                                qT[:, ec * OWNQ + s * QW + q0: ec * OWNQ + s * QW + QW],
                                start=(ec == 0), stop=(ec == NE - 1),
                            )
                        if t_idx >= 0:
                            nc.vector.tensor_add(
                                ps_sc[:, q0:QW], ps_sc[:, q0:QW],
                                mask_t[:, t_idx * QW + q0:(t_idx + 1) * QW],
                            )
                        pt = ptp.tile([128, QW], BF16, name="pt", tag="pt")
                        nc.scalar.activation(pt[:, q0:QW], ps_sc[:, q0:QW],
                                             EXP, scale=SCALE)
                        pending.append((s, kb, pt, ctx_cur, dn_cur))
                        if len(pending) > DEPTH:
                            consume(pending.popleft())
                while pending:
                    consume(pending.popleft())

    nc.compile()
    return nc


def _get_nc():
    if "nc" not in _cached:
        _cached["nc"] = _build()
    return _cached["nc"]


def build_in_maps(x, W_q, W_k, W_v):
    import ml_dtypes

    BF = ml_dtypes.bfloat16
    x = np.asarray(x, dtype=np.float32)
    wq = np.ascontiguousarray(np.asarray(W_q, dtype=BF))
    wk = np.ascontiguousarray(np.asarray(W_k, dtype=BF))
    wv = np.ascontiguousarray(np.asarray(W_v, dtype=BF))
    warm = np.ones((128, 640), dtype=BF)

    p = np.arange(128, dtype=np.int64)[:, None]
    f = np.arange(QW, dtype=np.int64)[None, :]
    masks_h = []
    for h in range(2):
        tiles = [
            np.where(128 * t + p <= 2 * f + h, np.float32(0.0), np.float32(MASK_NEG))
            for t in range(4)
        ]
        masks_h.append(np.concatenate(tiles, axis=1).astype(np.float32))

    xt_b = [np.ascontiguousarray(x[b].T.astype(BF)) for b in range(B)]
    in_maps = []
    for c in range(8):
        b, h = c // 2, c % 2
        xq = np.ascontiguousarray(x[b, h::2, :].T.astype(BF))
        in_maps.append({
            "xt": xt_b[b],
            "xq": xq,
            "wq": wq,
            "wk": wk,
            "wv": wv,
            "masks": masks_h[h],
            "warm": warm,
        })
    return in_maps


def kernel(x, W_q, W_k, W_v):
    from concourse.bass_utils import run_bass_kernel_spmd

    in_maps = build_in_maps(x, W_q, W_k, W_v)
    nc = _get_nc()
    res = run_bass_kernel_spmd(nc, in_maps, core_ids=list(range(8)))

    out = np.empty((B, S, D), dtype=np.float32)
    for c in range(8):
        b, h = c // 2, c % 2
        out[b, h::2, :] = res.results[c]["o"]
    return out


# revision 8
# speedup vs baseline: 1.2665x; 1.0523x over previous
"""Causal attention (B=4, S=2048, D=1024, fp32 in/out) on 8 Trainium2 cores.

Sharding: data-parallel over batch (4) x query-split (2) per batch. The two
cores of a batch take interleaved query rows (even/odd within each 512-row
super-block), which makes the causal workload identical on every core and
lets one SPMD program serve all 8 cores; the only per-core differences are
pure data (which query columns of x^T each core receives, and the mask
tiles, which carry the even/odd offset).

v2 changes vs the 300us baseline:
  - bf16 activations/weights (host-cast; fp32 PSUM accumulate everywhere).
    Halves HBM+SBUF traffic and drops LDWEIGHTS to ~53ns so weight loads
    hide fully under the N=512 matmul stream.
  - Projection restructured to a single pass over x with the full (bf16)
    weight matrices resident: all proj matmuls are N=512 (640 MMs instead
    of 1024 mixed N=256/512) -> per-MM overhead and LDW exposure drop.
  - Warm-up matmuls at t=0 keep the PE HAM clock-gate warm while the first
    DMAs stream, killing the 17us cold-start gap.
  - Normalize muls split across vector+scalar so slot-boundary drains don't
    queue behind the mask-add/exp chain; DEPTH=3 consume pipeline.

Per core:
  kT = (x @ W_k)^T, v = x @ W_v, qT = (x_own @ W_q)^T   (bf16 matmuls)
  For each of 4 query slots s (256 queries from super-block [512s, 512s+512)):
    for key block kb in [0, 4s+4): scoresT = kT_blk^T q -> +mask -> exp
      (no max-subtraction: scaled scores are ~N(0,1), exp is fp32-safe)
      denominators via ones-matmul; ctx accumulation in PSUM
    normalize by reciprocal(denom), DMA out (fp32).
"""

import numpy as np

B, S, D = 4, 2048, 1024
NE = D // 128          # contraction chunks (d on partitions)
NKBLK = S // 128       # 128-wide key blocks
NSLOT = 4              # query slots per core
QW = 256               # queries per slot
OWNQ = NSLOT * QW      # 1024 queries per core
MASK_NEG = -1.0e30
SCALE = 1.0 / 32.0     # 1/sqrt(D)
NWARM = 20             # PE warm-up matmuls at kernel start

_cached = {}


def _build():
    import concourse.bacc as bacc
    import concourse.tile as tile
    import concourse.mybir as mybir

    F32 = mybir.dt.float32
    BF16 = mybir.dt.bfloat16
    EXP = mybir.ActivationFunctionType.Exp

    nc = bacc.Bacc("TRN2", target_bir_lowering=False, debug=False, num_devices=8,
                   dynamic_dma_scratch_size=2048)

    xt_d = nc.dram_tensor("xt", [D, S], BF16, kind="ExternalInput")
    xq_d = nc.dram_tensor("xq", [D, OWNQ], BF16, kind="ExternalInput")
    wq_d = nc.dram_tensor("wq", [D, D], BF16, kind="ExternalInput")
    wk_d = nc.dram_tensor("wk", [D, D], BF16, kind="ExternalInput")
    wv_d = nc.dram_tensor("wv", [D, D], BF16, kind="ExternalInput")
    mask_d = nc.dram_tensor("masks", [128, 4 * QW], F32, kind="ExternalInput")
    warm_d = nc.dram_tensor("warm", [128, 640], BF16, kind="ExternalInput")
    o_d = nc.dram_tensor("o", [OWNQ, D], F32, kind="ExternalOutput")

    with tile.TileContext(nc) as tc:
        with tc.tile_pool(name="res", bufs=1) as res:
            # column layouts: kT chunk ec at [ec*S + key], v block kb at
            # [kb*D + dout], qT chunk ec at [ec*OWNQ + q]
            kT = res.tile([128, NE * S], BF16, name="kT", tag="kT")
            vv = res.tile([128, NKBLK * D], BF16, name="vv", tag="vv")
            qT = res.tile([128, NE * OWNQ], BF16, name="qT", tag="qT")
            warm_t = res.tile([128, 640], BF16, name="warm_t", tag="warm_t")
            mask_t = res.tile([128, 4 * QW], F32, name="mask_t", tag="mask_t")
            nc.sync.dma_start(warm_t[:, :], warm_d[:, :])
            nc.scalar.dma_start(mask_t[:, :], mask_d[:, :])

            # ---------------- projection phase ----------------
            with (
                tc.tile_pool(name="wpool", bufs=3) as wpool,
                tc.tile_pool(name="xpool", bufs=1) as xpool,
                tc.tile_pool(name="pp", bufs=6, space="PSUM") as pp,
                tc.tile_pool(name="wmp", bufs=2, space="PSUM") as wmp,
            ):
                # Warm-up matmuls on the (tiny, early-arriving) warm tile:
                # keeps PE activity up while the big input DMAs stream, so
                # the HAM clock-gate reaches 2.4GHz before real work starts.
                for i in range(NWARM):
                    wps = wmp.tile([128, 256], F32, name="wps", tag="wps")
                    nc.tensor.matmul(wps[:, :], warm_t[:, 0:128],
                                     warm_t[:, 128:384], start=True, stop=True)

                def load_w(dst, src):
                    # full [D, D] weight as 8 chunk DMAs over the two HWDGE
                    # queues (sync+scalar). The DMA instruction occupies the
                    # issuing engine's FIFO slot for the whole transfer, so
                    # every PSUM->SBUF copy lives on vector (which issues no
                    # DMAs) -- otherwise the copies queue behind the input
                    # stream and the proj pipeline stalls on PSUM slots.
                    for dc in range(NE):
                        eng = nc.sync if dc % 2 == 0 else nc.scalar
                        eng.dma_start(
                            dst[:, dc * D:(dc + 1) * D],
                            src[dc * 128:(dc + 1) * 128, :],
                        )

                wk_t = wpool.tile([128, NE * D], BF16, name="wk_t", tag="w")
                xt_t = xpool.tile([128, NE * S], BF16, name="xt_t", tag="xt")
                # DMA order tracks first use: (wk chunk dc + xt h0 chunk dc)
                # pairs feed the first K groups ASAP, then wv (V starts after
                # one K block), then xt h1 (used from js=2), then wq, xq.
                def load_xt_half(h):
                    for dc in range(NE):
                        eng = nc.sync if dc % 2 == 0 else nc.scalar
                        eng.dma_start(
                            xt_t[:, dc * S + h * 1024: dc * S + (h + 1) * 1024],
                            xt_d[dc * 128:(dc + 1) * 128, h * 1024:(h + 1) * 1024],
                        )

                for dc in range(NE):
                    eng = nc.sync if dc % 2 == 0 else nc.scalar
                    eng.dma_start(wk_t[:, dc * D:(dc + 1) * D],
                                  wk_d[dc * 128:(dc + 1) * 128, :])
                    eng2 = nc.scalar if dc % 2 == 0 else nc.sync
                    eng2.dma_start(
                        xt_t[:, dc * S: dc * S + 1024],
                        xt_d[dc * 128:(dc + 1) * 128, 0:1024],
                    )
                wv_t = wpool.tile([128, NE * D], BF16, name="wv_t", tag="w")
                load_w(wv_t, wv_d)
                load_xt_half(1)
                wq_t = wpool.tile([128, NE * D], BF16, name="wq_t", tag="w")
                load_w(wq_t, wq_d)
                xq_t = xpool.tile([128, NE * OWNQ], BF16, name="xq_t", tag="xq")
                for dc in range(NE):
                    eng = nc.sync if dc % 2 == 0 else nc.scalar
                    eng.dma_start(
                        xq_t[:, dc * OWNQ:(dc + 1) * OWNQ],
                        xq_d[dc * 128:(dc + 1) * 128, :],
                    )

                # K + V in one pass over the resident x^T; all MMs N=512.
                for js in range(S // 512):
                    for ei in range(NE):
                        ps = pp.tile([128, 512], F32, name="ps_p", tag="ps_p")
                        for dc in range(NE):
                            nc.tensor.matmul(
                                ps[:, :],
                                wk_t[:, dc * D + ei * 128: dc * D + (ei + 1) * 128],
                                xt_t[:, dc * S + js * 512: dc * S + (js + 1) * 512],
                                start=(dc == 0), stop=(dc == NE - 1),
                            )
                        nc.vector.tensor_copy(
                            kT[:, ei * S + js * 512: ei * S + (js + 1) * 512],
                            ps[:, :])
                    for jt in range(4):
                        kb = 4 * js + jt
                        for dh in range(2):
                            ps = pp.tile([128, 512], F32, name="ps_p", tag="ps_p")
                            for dc in range(NE):
                                nc.tensor.matmul(
                                    ps[:, :],
                                    xt_t[:, dc * S + kb * 128: dc * S + kb * 128 + 128],
                                    wv_t[:, dc * D + dh * 512: dc * D + (dh + 1) * 512],
                                    start=(dc == 0), stop=(dc == NE - 1),
                                )
                            nc.vector.tensor_copy(
                                vv[:, kb * D + dh * 512: kb * D + (dh + 1) * 512],
                                ps[:, :],
                            )

                # Q pass: qT[e, i] = sum_d Wq[d, e] xq[d, i]
                for isl in range(OWNQ // 512):
                    for ei in range(NE):
                        ps = pp.tile([128, 512], F32, name="ps_p", tag="ps_p")
                        for dc in range(NE):
                            nc.tensor.matmul(
                                ps[:, :],
                                wq_t[:, dc * D + ei * 128: dc * D + (ei + 1) * 128],
                                xq_t[:, dc * OWNQ + isl * 512: dc * OWNQ + (isl + 1) * 512],
                                start=(dc == 0), stop=(dc == NE - 1),
                            )
                        nc.vector.tensor_copy(
                            qT[:, ei * OWNQ + isl * 512: ei * OWNQ + (isl + 1) * 512],
                            ps[:, :])

            # ---------------- attention phase ----------------
            with (
                tc.tile_pool(name="ptp", bufs=6) as ptp,
                tc.tile_pool(name="obp", bufs=2) as obp,
                tc.tile_pool(name="rcp", bufs=2) as rcp,
                tc.tile_pool(name="scp", bufs=3, space="PSUM") as scp,
                tc.tile_pool(name="ctxp", bufs=1, space="PSUM") as ctxp,
                tc.tile_pool(name="dnp", bufs=1, space="PSUM") as dnp,
            ):
                def consume(item):
                    s, kb, pt, ctx, dn = item
                    nk = 4 * s + 4
                    t_idx = kb - (nk - 4)
                    # Diagonal trim: for diag block t, only queries f >= 64*t
                    # can attend (the rest are fully masked). Scores/exp were
                    # computed only on [q0, 256); dn/ctx must restrict their
                    # stationary (pt) columns the same way — pt[:, :q0] is
                    # uninitialized garbage.
                    q0 = 64 * t_idx if t_idx > 0 else 0
                    for c in range(2):
                        lo = max(q0, c * 128)
                        hi = (c + 1) * 128
                        if lo >= hi:
                            continue  # whole c-half masked for this block
                        # c=0 ends early on trimmed slots (its last two diag
                        # blocks are skipped): close its accumulation group
                        # on its last executed block.
                        last_kb = (nk - 3) if c == 0 else (nk - 1)
                        # Both column groups live in one PSUM bank; start=True
                        # clears the whole bank, so only the first group may
                        # set it — the second lands on freshly cleared psum
                        # (has_written=0) and still overwrites, not adds.
                        nc.tensor.matmul(
                            dn[lo - c * 128: hi - c * 128, 2 * c:2 * c + 2],
                            pt[:, lo:hi],
                            warm_t[:, 0:2],
                            start=(kb == 0 and c == 0), stop=(kb == last_kb),
                            skip_group_check=True,
                        )
                        for dh in range(2):
                            nc.tensor.matmul(
                                ctx[(c, dh)][lo - c * 128: hi - c * 128, :],
                                pt[:, lo:hi],
                                vv[:, kb * D + dh * 512: kb * D + (dh + 1) * 512],
                                start=(kb == 0), stop=(kb == last_kb),
                                skip_group_check=True,
                            )
                    if kb == nk - 1:
                        rc = rcp.tile([128, 2], F32, name="rc", tag="rc")
                        nc.vector.reciprocal(rc[:, :], dn[:, 0:4:2])
                        for c in range(2):
                            ob = obp.tile([128, D], F32, name="ob", tag="ob")
                            for dh in range(2):
                                # split the 4 normalize muls across vector +
                                # scalar so neither queue stalls the next
                                # slot's mask-add/exp chain
                                eng = nc.vector if c == 0 else nc.scalar
                                if eng is nc.vector:
                                    eng.tensor_scalar_mul(
                                        ob[:, dh * 512:(dh + 1) * 512],
                                        ctx[(c, dh)][:, :],
                                        rc[:, c:c + 1],
                                    )
                                else:
                                    eng.mul(
                                        ob[:, dh * 512:(dh + 1) * 512],
                                        ctx[(c, dh)][:, :],
                                        rc[:, c:c + 1],
                                    )
                            eng_d = nc.sync
                            eng_d.dma_start(
                                o_d[s * QW + c * 128: s * QW + (c + 1) * 128, :],
                                ob[:, :],
                            )

                from collections import deque
                pending = deque()
                DEPTH = 3
                for s in range(NSLOT):
                    nk = 4 * s + 4
                    # Drain before each slot: the slot's ctx/dn pool slots
                    # (bufs=1) can only be re-allocated once the previous
                    # slot's normalize has been emitted.
                    while pending:
                        consume(pending.popleft())
                    ctx_cur = {}
                    for c in range(2):
                        for dh in range(2):
                            t = ctxp.tile(
                                [128, 512], F32,
                                name=f"ctx{c}{dh}", tag=f"ctx{c}{dh}",
                            )
                            ctx_cur[(c, dh)] = t
                    dn_cur = dnp.tile([128, 4], F32, name="dn", tag="dn")
                    for kb in range(nk):
                        t_idx = kb - (nk - 4)
                        q0 = 64 * t_idx if t_idx > 0 else 0
                        qn = QW - q0
                        ps_sc = scp.tile([128, QW], F32, name="ps_sc", tag="sc")
                        for ec in range(NE):
                            nc.tensor.matmul(
                                ps_sc[:, q0:QW],
                                kT[:, ec * S + kb * 128: ec * S + (kb + 1) * 128],
                                qT[:, ec * OWNQ + s * QW + q0: ec * OWNQ + s * QW + QW],
                                start=(ec == 0), stop=(ec == NE - 1),
                            )
                        if t_idx >= 0:
                            nc.vector.tensor_add(
                                ps_sc[:, q0:QW], ps_sc[:, q0:QW],
                                mask_t[:, t_idx * QW + q0:(t_idx + 1) * QW],
                            )
                        pt = ptp.tile([128, QW], BF16, name="pt", tag="pt")
                        nc.scalar.activation(pt[:, q0:QW], ps_sc[:, q0:QW],
                                             EXP, scale=SCALE)
                        pending.append((s, kb, pt, ctx_cur, dn_cur))
                        if len(pending) > DEPTH:
                            consume(pending.popleft())
                while pending:
                    consume(pending.popleft())

    nc.compile()
    return nc


def _get_nc():
    if "nc" not in _cached:
        _cached["nc"] = _build()
    return _cached["nc"]


def build_in_maps(x, W_q, W_k, W_v):
    import ml_dtypes

    BF = ml_dtypes.bfloat16
    x = np.asarray(x, dtype=np.float32)
    wq = np.ascontiguousarray(np.asarray(W_q, dtype=BF))
    wk = np.ascontiguousarray(np.asarray(W_k, dtype=BF))
    wv = np.ascontiguousarray(np.asarray(W_v, dtype=BF))
    warm = np.ones((128, 640), dtype=BF)

    p = np.arange(128, dtype=np.int64)[:, None]
    f = np.arange(QW, dtype=np.int64)[None, :]
    masks_h = []
    for h in range(2):
        tiles = [
            np.where(128 * t + p <= 2 * f + h, np.float32(0.0), np.float32(MASK_NEG))
            for t in range(4)
        ]
        masks_h.append(np.concatenate(tiles, axis=1).astype(np.float32))

    xt_b = [np.ascontiguousarray(x[b].T.astype(BF)) for b in range(B)]
    in_maps = []
    for c in range(8):
        b, h = c // 2, c % 2
        xq = np.ascontiguousarray(x[b, h::2, :].T.astype(BF))
        in_maps.append({
            "xt": xt_b[b],
            "xq": xq,
            "wq": wq,
            "wk": wk,
            "wv": wv,
            "masks": masks_h[h],
            "warm": warm,
        })
    return in_maps


def kernel(x, W_q, W_k, W_v):
    from concourse.bass_utils import run_bass_kernel_spmd

    in_maps = build_in_maps(x, W_q, W_k, W_v)
    nc = _get_nc()
    res = run_bass_kernel_spmd(nc, in_maps, core_ids=list(range(8)))

    out = np.empty((B, S, D), dtype=np.float32)
    for c in range(8):
        b, h = c // 2, c % 2
        out[b, h::2, :] = res.results[c]["o"]
    return out


# revision 10
# speedup vs baseline: 1.2936x; 1.0214x over previous
"""Causal attention (B=4, S=2048, D=1024, fp32 in/out) on 8 Trainium2 cores.

Sharding: data-parallel over batch (4) x query-split (2) per batch. The two
cores of a batch take interleaved query rows (even/odd within each 512-row
super-block), which makes the causal workload identical on every core and
lets one SPMD program serve all 8 cores; the only per-core differences are
pure data (which query columns of x^T each core receives, and the mask
tiles, which carry the even/odd offset).

v2 changes vs the 300us baseline:
  - bf16 activations/weights (host-cast; fp32 PSUM accumulate everywhere).
    Halves HBM+SBUF traffic and drops LDWEIGHTS to ~53ns so weight loads
    hide fully under the N=512 matmul stream.
  - Projection restructured to a single pass over x with the full (bf16)
    weight matrices resident: all proj matmuls are N=512 (640 MMs instead
    of 1024 mixed N=256/512) -> per-MM overhead and LDW exposure drop.
  - Warm-up matmuls at t=0 keep the PE HAM clock-gate warm while the first
    DMAs stream, killing the 17us cold-start gap.
  - Normalize muls split across vector+scalar so slot-boundary drains don't
    queue behind the mask-add/exp chain; DEPTH=3 consume pipeline.

Per core:
  kT = (x @ W_k)^T, v = x @ W_v, qT = (x_own @ W_q)^T   (bf16 matmuls)
  For each of 4 query slots s (256 queries from super-block [512s, 512s+512)):
    for key block kb in [0, 4s+4): scoresT = kT_blk^T q -> +mask -> exp
      (no max-subtraction: scaled scores are ~N(0,1), exp is fp32-safe)
      denominators via ones-matmul; ctx accumulation in PSUM
    normalize by reciprocal(denom), DMA out (fp32).
"""

import numpy as np

B, S, D = 4, 2048, 1024
NE = D // 128          # contraction chunks (d on partitions)
NKBLK = S // 128       # 128-wide key blocks
NSLOT = 4              # query slots per core
QW = 256               # queries per slot
OWNQ = NSLOT * QW      # 1024 queries per core
MASK_NEG = -1.0e30
SCALE = 1.0 / 32.0     # 1/sqrt(D)
NWARM = 20             # PE warm-up matmuls at kernel start

_cached = {}


def _build():
    import concourse.bacc as bacc
    import concourse.tile as tile
    import concourse.mybir as mybir

    F32 = mybir.dt.float32
    BF16 = mybir.dt.bfloat16
    EXP = mybir.ActivationFunctionType.Exp

    nc = bacc.Bacc("TRN2", target_bir_lowering=False, debug=False, num_devices=8,
                   dynamic_dma_scratch_size=2048)

    xt_d = nc.dram_tensor("xt", [D, S], BF16, kind="ExternalInput")
    xq_d = nc.dram_tensor("xq", [D, OWNQ], BF16, kind="ExternalInput")
    wq_d = nc.dram_tensor("wq", [D, D], BF16, kind="ExternalInput")
    wk_d = nc.dram_tensor("wk", [D, D], BF16, kind="ExternalInput")
    wv_d = nc.dram_tensor("wv", [D, D], BF16, kind="ExternalInput")
    mask_d = nc.dram_tensor("masks", [128, 4 * QW], F32, kind="ExternalInput")
    warm_d = nc.dram_tensor("warm", [128, 640], BF16, kind="ExternalInput")
    o_d = nc.dram_tensor("o", [OWNQ, D], F32, kind="ExternalOutput")

    with tile.TileContext(nc) as tc:
        with tc.tile_pool(name="res", bufs=1) as res:
            # column layouts: kT chunk ec at [ec*S + key], v block kb at
            # [kb*D + dout], qT chunk ec at [ec*OWNQ + q]
            kT = res.tile([128, NE * S], BF16, name="kT", tag="kT")
            vv = res.tile([128, NKBLK * D], BF16, name="vv", tag="vv")
            qT = res.tile([128, NE * OWNQ], BF16, name="qT", tag="qT")
            warm_t = res.tile([128, 640], BF16, name="warm_t", tag="warm_t")
            mask_t = res.tile([128, 4 * QW], F32, name="mask_t", tag="mask_t")
            nc.sync.dma_start(warm_t[:, :], warm_d[:, :])

            # ---------------- projection phase ----------------
            with (
                tc.tile_pool(name="wpool", bufs=3) as wpool,
                tc.tile_pool(name="xpool", bufs=1) as xpool,
                tc.tile_pool(name="pp", bufs=6, space="PSUM") as pp,
                tc.tile_pool(name="wmp", bufs=2, space="PSUM") as wmp,
            ):
                # Warm-up matmuls on the (tiny, early-arriving) warm tile:
                # keeps PE activity up while the big input DMAs stream, so
                # the HAM clock-gate reaches 2.4GHz before real work starts.
                for i in range(NWARM):
                    wps = wmp.tile([128, 256], F32, name="wps", tag="wps")
                    nc.tensor.matmul(wps[:, :], warm_t[:, 0:128],
                                     warm_t[:, 128:384], start=True, stop=True)

                def load_w(dst, src):
                    # full [D, D] weight as 8 chunk DMAs over the two HWDGE
                    # queues (sync+scalar). The DMA instruction occupies the
                    # issuing engine's FIFO slot for the whole transfer, so
                    # every PSUM->SBUF copy lives on vector (which issues no
                    # DMAs) -- otherwise the copies queue behind the input
                    # stream and the proj pipeline stalls on PSUM slots.
                    for dc in range(NE):
                        eng = nc.sync if dc % 2 == 0 else nc.scalar
                        eng.dma_start(
                            dst[:, dc * D:(dc + 1) * D],
                            src[dc * 128:(dc + 1) * 128, :],
                        )

                wk_t = wpool.tile([128, NE * D], BF16, name="wk_t", tag="w")
                xt_t = xpool.tile([128, NE * S], BF16, name="xt_t", tag="xt")
                # DMA order tracks first use: (wk chunk dc + xt h0 chunk dc)
                # pairs feed the first K groups ASAP, then wv (V starts after
                # one K block), then xt h1 (used from js=2), then wq, xq.
                def load_xt_half(h):
                    for dc in range(NE):
                        eng = nc.sync if dc % 2 == 0 else nc.scalar
                        eng.dma_start(
                            xt_t[:, dc * S + h * 1024: dc * S + (h + 1) * 1024],
                            xt_d[dc * 128:(dc + 1) * 128, h * 1024:(h + 1) * 1024],
                        )

                def load_xt_cols(c0, c1):
                    for dc in range(NE):
                        eng = nc.sync if dc % 2 == 0 else nc.scalar
                        eng.dma_start(
                            xt_t[:, dc * S + c0: dc * S + c1],
                            xt_d[dc * 128:(dc + 1) * 128, c0:c1],
                        )

                # first K group needs wk + xt cols [0:512) only: pair those
                # so the PE ladder starts as early as the HBM stream allows
                for dc in range(NE):
                    eng = nc.sync if dc % 2 == 0 else nc.scalar
                    eng.dma_start(wk_t[:, dc * D:(dc + 1) * D],
                                  wk_d[dc * 128:(dc + 1) * 128, :])
                    eng2 = nc.scalar if dc % 2 == 0 else nc.sync
                    eng2.dma_start(
                        xt_t[:, dc * S: dc * S + 512],
                        xt_d[dc * 128:(dc + 1) * 128, 0:512],
                    )
                wv_t = wpool.tile([128, NE * D], BF16, name="wv_t", tag="w")
                load_w(wv_t, wv_d)
                load_xt_cols(512, 1024)
                load_xt_half(1)
                wq_t = wpool.tile([128, NE * D], BF16, name="wq_t", tag="w")
                load_w(wq_t, wq_d)
                xq_t = xpool.tile([128, NE * OWNQ], BF16, name="xq_t", tag="xq")
                for dc in range(NE):
                    eng = nc.sync if dc % 2 == 0 else nc.scalar
                    eng.dma_start(
                        xq_t[:, dc * OWNQ:(dc + 1) * OWNQ],
                        xq_d[dc * 128:(dc + 1) * 128, :],
                    )
                nc.scalar.dma_start(mask_t[:, :], mask_d[:, :])

                # K + V in one pass over the resident x^T; all MMs N=512.
                for js in range(S // 512):
                    for ei in range(NE):
                        ps = pp.tile([128, 512], F32, name="ps_p", tag="ps_p")
                        for dc in range(NE):
                            nc.tensor.matmul(
                                ps[:, :],
                                wk_t[:, dc * D + ei * 128: dc * D + (ei + 1) * 128],
                                xt_t[:, dc * S + js * 512: dc * S + (js + 1) * 512],
                                start=(dc == 0), stop=(dc == NE - 1),
                            )
                        nc.vector.tensor_copy(
                            kT[:, ei * S + js * 512: ei * S + (js + 1) * 512],
                            ps[:, :])
                    for jt in range(4):
                        kb = 4 * js + jt
                        for dh in range(2):
                            ps = pp.tile([128, 512], F32, name="ps_p", tag="ps_p")
                            for dc in range(NE):
                                nc.tensor.matmul(
                                    ps[:, :],
                                    xt_t[:, dc * S + kb * 128: dc * S + kb * 128 + 128],
                                    wv_t[:, dc * D + dh * 512: dc * D + (dh + 1) * 512],
                                    start=(dc == 0), stop=(dc == NE - 1),
                                )
                            nc.vector.tensor_copy(
                                vv[:, kb * D + dh * 512: kb * D + (dh + 1) * 512],
                                ps[:, :],
                            )

                # Q pass: qT[e, i] = sum_d Wq[d, e] xq[d, i]
                for isl in range(OWNQ // 512):
                    for ei in range(NE):
                        ps = pp.tile([128, 512], F32, name="ps_p", tag="ps_p")
                        for dc in range(NE):
                            nc.tensor.matmul(
                                ps[:, :],
                                wq_t[:, dc * D + ei * 128: dc * D + (ei + 1) * 128],
                                xq_t[:, dc * OWNQ + isl * 512: dc * OWNQ + (isl + 1) * 512],
                                start=(dc == 0), stop=(dc == NE - 1),
                            )
                        nc.vector.tensor_copy(
                            qT[:, ei * OWNQ + isl * 512: ei * OWNQ + (isl + 1) * 512],
                            ps[:, :])

            # ---------------- attention phase ----------------
            with (
                tc.tile_pool(name="ptp", bufs=6) as ptp,
                tc.tile_pool(name="obp", bufs=2) as obp,
                tc.tile_pool(name="rcp", bufs=2) as rcp,
                tc.tile_pool(name="scp", bufs=3, space="PSUM") as scp,
                tc.tile_pool(name="ctxp", bufs=1, space="PSUM") as ctxp,
                tc.tile_pool(name="dnp", bufs=1, space="PSUM") as dnp,
            ):
                def consume(item):
                    s, kb, pt, ctx, dn = item
                    nk = 4 * s + 4
                    t_idx = kb - (nk - 4)
                    # Diagonal trim: for diag block t, only queries f >= 64*t
                    # can attend (the rest are fully masked). Scores/exp were
                    # computed only on [q0, 256); dn/ctx must restrict their
                    # stationary (pt) columns the same way — pt[:, :q0] is
                    # uninitialized garbage.
                    q0 = 64 * t_idx if t_idx > 0 else 0
                    for c in range(2):
                        lo = max(q0, c * 128)
                        hi = (c + 1) * 128
                        if lo >= hi:
                            continue  # whole c-half masked for this block
                        # c=0 ends early on trimmed slots (its last two diag
                        # blocks are skipped): close its accumulation group
                        # on its last executed block.
                        last_kb = (nk - 3) if c == 0 else (nk - 1)
                        # Both column groups live in one PSUM bank; start=True
                        # clears the whole bank, so only the first group may
                        # set it — the second lands on freshly cleared psum
                        # (has_written=0) and still overwrites, not adds.
                        nc.tensor.matmul(
                            dn[lo - c * 128: hi - c * 128, 2 * c:2 * c + 2],
                            pt[:, lo:hi],
                            warm_t[:, 0:2],
                            start=(kb == 0 and c == 0), stop=(kb == last_kb),
                            skip_group_check=True,
                        )
                        for dh in range(2):
                            nc.tensor.matmul(
                                ctx[(c, dh)][lo - c * 128: hi - c * 128, :],
                                pt[:, lo:hi],
                                vv[:, kb * D + dh * 512: kb * D + (dh + 1) * 512],
                                start=(kb == 0), stop=(kb == last_kb),
                                skip_group_check=True,
                            )
                    if kb == nk - 1:
                        rc = rcp.tile([128, 2], F32, name="rc", tag="rc")
                        nc.vector.reciprocal(rc[:, :], dn[:, 0:4:2])
                        for c in range(2):
                            ob = obp.tile([128, D], F32, name="ob", tag="ob")
                            for dh in range(2):
                                # split the 4 normalize muls across vector +
                                # scalar so neither queue stalls the next
                                # slot's mask-add/exp chain
                                eng = nc.vector if c == 0 else nc.scalar
                                if eng is nc.vector:
                                    eng.tensor_scalar_mul(
                                        ob[:, dh * 512:(dh + 1) * 512],
                                        ctx[(c, dh)][:, :],
                                        rc[:, c:c + 1],
                                    )
                                else:
                                    eng.mul(
                                        ob[:, dh * 512:(dh + 1) * 512],
                                        ctx[(c, dh)][:, :],
                                        rc[:, c:c + 1],
                                    )
                            eng_d = nc.sync if c == 0 else nc.scalar
                            eng_d.dma_start(
                                o_d[s * QW + c * 128: s * QW + (c + 1) * 128, :],
                                ob[:, :],
                            )

                from collections import deque
                pending = deque()
                DEPTH = 3
                for s in range(NSLOT):
                    nk = 4 * s + 4
                    # Drain before each slot: the slot's ctx/dn pool slots
                    # (bufs=1) can only be re-allocated once the previous
                    # slot's normalize has been emitted.
                    while pending:
                        consume(pending.popleft())
                    ctx_cur = {}
                    for c in range(2):
                        for dh in range(2):
                            t = ctxp.tile(
                                [128, 512], F32,
                                name=f"ctx{c}{dh}", tag=f"ctx{c}{dh}",
                            )
                            ctx_cur[(c, dh)] = t
                    dn_cur = dnp.tile([128, 4], F32, name="dn", tag="dn")
                    for kb in range(nk):
                        t_idx = kb - (nk - 4)
                        q0 = 64 * t_idx if t_idx > 0 else 0
                        qn = QW - q0
                        ps_sc = scp.tile([128, QW], F32, name="ps_sc", tag="sc")
                        for ec in range(NE):
                            nc.tensor.matmul(
                                ps_sc[:, q0:QW],
                                kT[:, ec * S + kb * 128: ec * S + (kb + 1) * 128],
                                qT[:, ec * OWNQ + s * QW + q0: ec * OWNQ + s * QW + QW],
                                start=(ec == 0), stop=(ec == NE - 1),
                            )
                        if t_idx >= 0:
                            nc.vector.tensor_add(
                                ps_sc[:, q0:QW], ps_sc[:, q0:QW],
                                mask_t[:, t_idx * QW + q0:(t_idx + 1) * QW],
                            )
                        pt = ptp.tile([128, QW], BF16, name="pt", tag="pt")
                        nc.scalar.activation(pt[:, q0:QW], ps_sc[:, q0:QW],
                                             EXP, scale=SCALE)
                        pending.append((s, kb, pt, ctx_cur, dn_cur))
                        if len(pending) > DEPTH:
                            consume(pending.popleft())
                while pending:
                    consume(pending.popleft())

    nc.compile()
    return nc


def _get_nc():
    if "nc" not in _cached:
        _cached["nc"] = _build()
    return _cached["nc"]


def build_in_maps(x, W_q, W_k, W_v):
    import ml_dtypes

    BF = ml_dtypes.bfloat16
    x = np.asarray(x, dtype=np.float32)
    wq = np.ascontiguousarray(np.asarray(W_q, dtype=BF))
    wk = np.ascontiguousarray(np.asarray(W_k, dtype=BF))
    wv = np.ascontiguousarray(np.asarray(W_v, dtype=BF))
    warm = np.ones((128, 640), dtype=BF)

    p = np.arange(128, dtype=np.int64)[:, None]
    f = np.arange(QW, dtype=np.int64)[None, :]
    masks_h = []
    for h in range(2):
        tiles = [
            np.where(128 * t + p <= 2 * f + h, np.float32(0.0), np.float32(MASK_NEG))
            for t in range(4)
        ]
        masks_h.append(np.concatenate(tiles, axis=1).astype(np.float32))

    xt_b = [np.ascontiguousarray(x[b].T.astype(BF)) for b in range(B)]
    in_maps = []
    for c in range(8):
        b, h = c // 2, c % 2
        xq = np.ascontiguousarray(x[b, h::2, :].T.astype(BF))
        in_maps.append({
            "xt": xt_b[b],
            "xq": xq,
            "wq": wq,
            "wk": wk,
            "wv": wv,
            "masks": masks_h[h],
            "warm": warm,
        })
    return in_maps


def kernel(x, W_q, W_k, W_v):
    from concourse.bass_utils import run_bass_kernel_spmd

    in_maps = build_in_maps(x, W_q, W_k, W_v)
    nc = _get_nc()
    res = run_bass_kernel_spmd(nc, in_maps, core_ids=list(range(8)))

    out = np.empty((B, S, D), dtype=np.float32)
    for c in range(8):
        b, h = c // 2, c % 2
        out[b, h::2, :] = res.results[c]["o"]
    return out


# revision 11
# speedup vs baseline: 1.3029x; 1.0071x over previous
"""Causal attention (B=4, S=2048, D=1024, fp32 in/out) on 8 Trainium2 cores.

Sharding: data-parallel over batch (4) x query-split (2) per batch. The two
cores of a batch take interleaved query rows (even/odd within each 512-row
super-block), which makes the causal workload identical on every core and
lets one SPMD program serve all 8 cores; the only per-core differences are
pure data (which query columns of x^T each core receives, and the mask
tiles, which carry the even/odd offset).

v2 changes vs the 300us baseline:
  - bf16 activations/weights (host-cast; fp32 PSUM accumulate everywhere).
    Halves HBM+SBUF traffic and drops LDWEIGHTS to ~53ns so weight loads
    hide fully under the N=512 matmul stream.
  - Projection restructured to a single pass over x with the full (bf16)
    weight matrices resident: all proj matmuls are N=512 (640 MMs instead
    of 1024 mixed N=256/512) -> per-MM overhead and LDW exposure drop.
  - Warm-up matmuls at t=0 keep the PE HAM clock-gate warm while the first
    DMAs stream, killing the 17us cold-start gap.
  - Normalize muls split across vector+scalar so slot-boundary drains don't
    queue behind the mask-add/exp chain; DEPTH=3 consume pipeline.

Per core:
  kT = (x @ W_k)^T, v = x @ W_v, qT = (x_own @ W_q)^T   (bf16 matmuls)
  For each of 4 query slots s (256 queries from super-block [512s, 512s+512)):
    for key block kb in [0, 4s+4): scoresT = kT_blk^T q -> +mask -> exp
      (no max-subtraction: scaled scores are ~N(0,1), exp is fp32-safe)
      denominators via ones-matmul; ctx accumulation in PSUM
    normalize by reciprocal(denom), DMA out (fp32).
"""

import numpy as np

B, S, D = 4, 2048, 1024
NE = D // 128          # contraction chunks (d on partitions)
NKBLK = S // 128       # 128-wide key blocks
NSLOT = 4              # query slots per core
QW = 256               # queries per slot
OWNQ = NSLOT * QW      # 1024 queries per core
MASK_NEG = -1.0e30
SCALE = 1.0 / 32.0     # 1/sqrt(D)
NWARM = 16             # PE warm-up matmuls at kernel start

_cached = {}


def _build():
    import concourse.bacc as bacc
    import concourse.tile as tile
    import concourse.mybir as mybir

    F32 = mybir.dt.float32
    BF16 = mybir.dt.bfloat16
    EXP = mybir.ActivationFunctionType.Exp

    nc = bacc.Bacc("TRN2", target_bir_lowering=False, debug=False, num_devices=8,
                   dynamic_dma_scratch_size=2048)

    xt_d = nc.dram_tensor("xt", [D, S], BF16, kind="ExternalInput")
    xq_d = nc.dram_tensor("xq", [D, OWNQ], BF16, kind="ExternalInput")
    wq_d = nc.dram_tensor("wq", [D, D], BF16, kind="ExternalInput")
    wk_d = nc.dram_tensor("wk", [D, D], BF16, kind="ExternalInput")
    wv_d = nc.dram_tensor("wv", [D, D], BF16, kind="ExternalInput")
    mask_d = nc.dram_tensor("masks", [128, 4 * QW], F32, kind="ExternalInput")
    warm_d = nc.dram_tensor("warm", [128, 640], BF16, kind="ExternalInput")
    o_d = nc.dram_tensor("o", [OWNQ, D], F32, kind="ExternalOutput")

    with tile.TileContext(nc) as tc:
        with tc.tile_pool(name="res", bufs=1) as res:
            # column layouts: kT chunk ec at [ec*S + key], v block kb at
            # [kb*D + dout], qT chunk ec at [ec*OWNQ + q]
            kT = res.tile([128, NE * S], BF16, name="kT", tag="kT")
            vv = res.tile([128, NKBLK * D], BF16, name="vv", tag="vv")
            qT = res.tile([128, NE * OWNQ], BF16, name="qT", tag="qT")
            warm_t = res.tile([128, 640], BF16, name="warm_t", tag="warm_t")
            mask_t = res.tile([128, 4 * QW], F32, name="mask_t", tag="mask_t")
            nc.sync.dma_start(warm_t[:, :], warm_d[:, :])

            # ---------------- projection phase ----------------
            with (
                tc.tile_pool(name="wpool", bufs=3) as wpool,
                tc.tile_pool(name="xpool", bufs=1) as xpool,
                tc.tile_pool(name="pp", bufs=6, space="PSUM") as pp,
                tc.tile_pool(name="wmp", bufs=2, space="PSUM") as wmp,
            ):
                # Warm-up matmuls on the (tiny, early-arriving) warm tile:
                # keeps PE activity up while the big input DMAs stream, so
                # the HAM clock-gate reaches 2.4GHz before real work starts.
                for i in range(NWARM):
                    wps = wmp.tile([128, 256], F32, name="wps", tag="wps")
                    nc.tensor.matmul(wps[:, :], warm_t[:, 0:128],
                                     warm_t[:, 128:384], start=True, stop=True)

                def load_w(dst, src):
                    # full [D, D] weight as 8 chunk DMAs over the two HWDGE
                    # queues (sync+scalar). The DMA instruction occupies the
                    # issuing engine's FIFO slot for the whole transfer, so
                    # every PSUM->SBUF copy lives on vector (which issues no
                    # DMAs) -- otherwise the copies queue behind the input
                    # stream and the proj pipeline stalls on PSUM slots.
                    for dc in range(NE):
                        eng = nc.sync if dc % 2 == 0 else nc.scalar
                        eng.dma_start(
                            dst[:, dc * D:(dc + 1) * D],
                            src[dc * 128:(dc + 1) * 128, :],
                        )

                wk_t = wpool.tile([128, NE * D], BF16, name="wk_t", tag="w")
                xt_t = xpool.tile([128, NE * S], BF16, name="xt_t", tag="xt")
                # DMA order tracks first use: (wk chunk dc + xt h0 chunk dc)
                # pairs feed the first K groups ASAP, then wv (V starts after
                # one K block), then xt h1 (used from js=2), then wq, xq.
                def load_xt_half(h):
                    for dc in range(NE):
                        eng = nc.sync if dc % 2 == 0 else nc.scalar
                        eng.dma_start(
                            xt_t[:, dc * S + h * 1024: dc * S + (h + 1) * 1024],
                            xt_d[dc * 128:(dc + 1) * 128, h * 1024:(h + 1) * 1024],
                        )

                def load_xt_cols(c0, c1):
                    for dc in range(NE):
                        eng = nc.sync if dc % 2 == 0 else nc.scalar
                        eng.dma_start(
                            xt_t[:, dc * S + c0: dc * S + c1],
                            xt_d[dc * 128:(dc + 1) * 128, c0:c1],
                        )

                # first K group needs wk + xt cols [0:512) only: pair those
                # so the PE ladder starts as early as the HBM stream allows
                for dc in range(NE):
                    eng = nc.sync if dc % 2 == 0 else nc.scalar
                    eng.dma_start(wk_t[:, dc * D:(dc + 1) * D],
                                  wk_d[dc * 128:(dc + 1) * 128, :])
                    eng2 = nc.scalar if dc % 2 == 0 else nc.sync
                    eng2.dma_start(
                        xt_t[:, dc * S: dc * S + 512],
                        xt_d[dc * 128:(dc + 1) * 128, 0:512],
                    )
                wv_t = wpool.tile([128, NE * D], BF16, name="wv_t", tag="w")
                load_w(wv_t, wv_d)
                load_xt_cols(512, 1024)
                load_xt_half(1)
                wq_t = wpool.tile([128, NE * D], BF16, name="wq_t", tag="w")
                load_w(wq_t, wq_d)
                xq_t = xpool.tile([128, NE * OWNQ], BF16, name="xq_t", tag="xq")
                for dc in range(NE):
                    eng = nc.sync if dc % 2 == 0 else nc.scalar
                    eng.dma_start(
                        xq_t[:, dc * OWNQ:(dc + 1) * OWNQ],
                        xq_d[dc * 128:(dc + 1) * 128, :],
                    )
                nc.scalar.dma_start(mask_t[:, :], mask_d[:, :])

                # K pass over the resident x^T; all MMs N=512.
                for js in range(S // 512):
                    for ei in range(NE):
                        ps = pp.tile([128, 512], F32, name="ps_p", tag="ps_p")
                        for dc in range(NE):
                            nc.tensor.matmul(
                                ps[:, :],
                                wk_t[:, dc * D + ei * 128: dc * D + (ei + 1) * 128],
                                xt_t[:, dc * S + js * 512: dc * S + (js + 1) * 512],
                                start=(dc == 0), stop=(dc == NE - 1),
                            )
                        nc.vector.tensor_copy(
                            kT[:, ei * S + js * 512: ei * S + (js + 1) * 512],
                            ps[:, :])

                # Q pass before V so that attention's first score groups
                # (which need kT+qT only) have their inputs long-finished by
                # the time the PE drains the V stream -- no boundary bubble.
                for isl in range(OWNQ // 512):
                    for ei in range(NE):
                        ps = pp.tile([128, 512], F32, name="ps_p", tag="ps_p")
                        for dc in range(NE):
                            nc.tensor.matmul(
                                ps[:, :],
                                wq_t[:, dc * D + ei * 128: dc * D + (ei + 1) * 128],
                                xq_t[:, dc * OWNQ + isl * 512: dc * OWNQ + (isl + 1) * 512],
                                start=(dc == 0), stop=(dc == NE - 1),
                            )
                        nc.vector.tensor_copy(
                            qT[:, ei * OWNQ + isl * 512: ei * OWNQ + (isl + 1) * 512],
                            ps[:, :])

                # V pass
                for kb in range(NKBLK):
                    for dh in range(2):
                        ps = pp.tile([128, 512], F32, name="ps_p", tag="ps_p")
                        for dc in range(NE):
                            nc.tensor.matmul(
                                ps[:, :],
                                xt_t[:, dc * S + kb * 128: dc * S + kb * 128 + 128],
                                wv_t[:, dc * D + dh * 512: dc * D + (dh + 1) * 512],
                                start=(dc == 0), stop=(dc == NE - 1),
                            )
                        nc.vector.tensor_copy(
                            vv[:, kb * D + dh * 512: kb * D + (dh + 1) * 512],
                            ps[:, :],
                        )

            # ---------------- attention phase ----------------
            with (
                tc.tile_pool(name="ptp", bufs=6) as ptp,
                tc.tile_pool(name="obp", bufs=2) as obp,
                tc.tile_pool(name="rcp", bufs=2) as rcp,
                tc.tile_pool(name="scp", bufs=3, space="PSUM") as scp,
                tc.tile_pool(name="ctxp", bufs=1, space="PSUM") as ctxp,
                tc.tile_pool(name="dnp", bufs=1, space="PSUM") as dnp,
            ):
                def consume(item):
                    s, kb, pt, ctx, dn = item
                    nk = 4 * s + 4
                    t_idx = kb - (nk - 4)
                    # Diagonal trim: for diag block t, only queries f >= 64*t
                    # can attend (the rest are fully masked). Scores/exp were
                    # computed only on [q0, 256); dn/ctx must restrict their
                    # stationary (pt) columns the same way — pt[:, :q0] is
                    # uninitialized garbage.
                    q0 = 64 * t_idx if t_idx > 0 else 0
                    for c in range(2):
                        lo = max(q0, c * 128)
                        hi = (c + 1) * 128
                        if lo >= hi:
                            continue  # whole c-half masked for this block
                        # c=0 ends early on trimmed slots (its last two diag
                        # blocks are skipped): close its accumulation group
                        # on its last executed block.
                        last_kb = (nk - 3) if c == 0 else (nk - 1)
                        # Both column groups live in one PSUM bank; start=True
                        # clears the whole bank, so only the first group may
                        # set it — the second lands on freshly cleared psum
                        # (has_written=0) and still overwrites, not adds.
                        nc.tensor.matmul(
                            dn[lo - c * 128: hi - c * 128, 2 * c:2 * c + 2],
                            pt[:, lo:hi],
                            warm_t[:, 0:2],
                            start=(kb == 0 and c == 0), stop=(kb == last_kb),
                            skip_group_check=True,
                        )
                        for dh in range(2):
                            nc.tensor.matmul(
                                ctx[(c, dh)][lo - c * 128: hi - c * 128, :],
                                pt[:, lo:hi],
                                vv[:, kb * D + dh * 512: kb * D + (dh + 1) * 512],
                                start=(kb == 0), stop=(kb == last_kb),
                                skip_group_check=True,
                            )
                    if kb == nk - 1:
                        rc = rcp.tile([128, 2], F32, name="rc", tag="rc")
                        nc.vector.reciprocal(rc[:, :], dn[:, 0:4:2])
                        for c in range(2):
                            ob = obp.tile([128, D], F32, name="ob", tag="ob")
                            for dh in range(2):
                                # split the 4 normalize muls across vector +
                                # scalar so neither queue stalls the next
                                # slot's mask-add/exp chain
                                eng = nc.vector if c == 0 else nc.scalar
                                if eng is nc.vector:
                                    eng.tensor_scalar_mul(
                                        ob[:, dh * 512:(dh + 1) * 512],
                                        ctx[(c, dh)][:, :],
                                        rc[:, c:c + 1],
                                    )
                                else:
                                    eng.mul(
                                        ob[:, dh * 512:(dh + 1) * 512],
                                        ctx[(c, dh)][:, :],
                                        rc[:, c:c + 1],
                                    )
                            eng_d = nc.sync if c == 0 else nc.scalar
                            eng_d.dma_start(
                                o_d[s * QW + c * 128: s * QW + (c + 1) * 128, :],
                                ob[:, :],
                            )

                from collections import deque
                pending = deque()
                DEPTH = 3
                for s in range(NSLOT):
                    nk = 4 * s + 4
                    # Drain before each slot: the slot's ctx/dn pool slots
                    # (bufs=1) can only be re-allocated once the previous
                    # slot's normalize has been emitted.
                    while pending:
                        consume(pending.popleft())
                    ctx_cur = {}
                    for c in range(2):
                        for dh in range(2):
                            t = ctxp.tile(
                                [128, 512], F32,
                                name=f"ctx{c}{dh}", tag=f"ctx{c}{dh}",
                            )
                            ctx_cur[(c, dh)] = t
                    dn_cur = dnp.tile([128, 4], F32, name="dn", tag="dn")
                    for kb in range(nk):
                        t_idx = kb - (nk - 4)
                        q0 = 64 * t_idx if t_idx > 0 else 0
                        qn = QW - q0
                        ps_sc = scp.tile([128, QW], F32, name="ps_sc", tag="sc")
                        for ec in range(NE):
                            nc.tensor.matmul(
                                ps_sc[:, q0:QW],
                                kT[:, ec * S + kb * 128: ec * S + (kb + 1) * 128],
                                qT[:, ec * OWNQ + s * QW + q0: ec * OWNQ + s * QW + QW],
                                start=(ec == 0), stop=(ec == NE - 1),
                            )
                        if t_idx >= 0:
                            nc.vector.tensor_add(
                                ps_sc[:, q0:QW], ps_sc[:, q0:QW],
                                mask_t[:, t_idx * QW + q0:(t_idx + 1) * QW],
                            )
                        pt = ptp.tile([128, QW], BF16, name="pt", tag="pt")
                        nc.scalar.activation(pt[:, q0:QW], ps_sc[:, q0:QW],
                                             EXP, scale=SCALE)
                        pending.append((s, kb, pt, ctx_cur, dn_cur))
                        if len(pending) > DEPTH:
                            consume(pending.popleft())
                while pending:
                    consume(pending.popleft())

    nc.compile()
    return nc


def _get_nc():
    if "nc" not in _cached:
        _cached["nc"] = _build()
    return _cached["nc"]


def build_in_maps(x, W_q, W_k, W_v):
    import ml_dtypes

    BF = ml_dtypes.bfloat16
    x = np.asarray(x, dtype=np.float32)
    wq = np.ascontiguousarray(np.asarray(W_q, dtype=BF))
    wk = np.ascontiguousarray(np.asarray(W_k, dtype=BF))
    wv = np.ascontiguousarray(np.asarray(W_v, dtype=BF))
    warm = np.ones((128, 640), dtype=BF)

    p = np.arange(128, dtype=np.int64)[:, None]
    f = np.arange(QW, dtype=np.int64)[None, :]
    masks_h = []
    for h in range(2):
        tiles = [
            np.where(128 * t + p <= 2 * f + h, np.float32(0.0), np.float32(MASK_NEG))
            for t in range(4)
        ]
        masks_h.append(np.concatenate(tiles, axis=1).astype(np.float32))

    xt_b = [np.ascontiguousarray(x[b].T.astype(BF)) for b in range(B)]
    in_maps = []
    for c in range(8):
        b, h = c // 2, c % 2
        xq = np.ascontiguousarray(x[b, h::2, :].T.astype(BF))
        in_maps.append({
            "xt": xt_b[b],
            "xq": xq,
            "wq": wq,
            "wk": wk,
            "wv": wv,
            "masks": masks_h[h],
            "warm": warm,
        })
    return in_maps


def kernel(x, W_q, W_k, W_v):
    from concourse.bass_utils import run_bass_kernel_spmd

    in_maps = build_in_maps(x, W_q, W_k, W_v)
    nc = _get_nc()
    res = run_bass_kernel_spmd(nc, in_maps, core_ids=list(range(8)))

    out = np.empty((B, S, D), dtype=np.float32)
    for c in range(8):
        b, h = c // 2, c % 2
        out[b, h::2, :] = res.results[c]["o"]
    return out


# revision 12
# speedup vs baseline: 1.3116x; 1.0067x over previous
"""Causal attention (B=4, S=2048, D=1024, fp32 in/out) on 8 Trainium2 cores.

Sharding: data-parallel over batch (4) x query-split (2) per batch. The two
cores of a batch take interleaved query rows (even/odd within each 512-row
super-block), which makes the causal workload identical on every core and
lets one SPMD program serve all 8 cores; the only per-core differences are
pure data (which query columns of x^T each core receives, and the mask
tiles, which carry the even/odd offset).

v2 changes vs the 300us baseline:
  - bf16 activations/weights (host-cast; fp32 PSUM accumulate everywhere).
    Halves HBM+SBUF traffic and drops LDWEIGHTS to ~53ns so weight loads
    hide fully under the N=512 matmul stream.
  - Projection restructured to a single pass over x with the full (bf16)
    weight matrices resident: all proj matmuls are N=512 (640 MMs instead
    of 1024 mixed N=256/512) -> per-MM overhead and LDW exposure drop.
  - Warm-up matmuls at t=0 keep the PE HAM clock-gate warm while the first
    DMAs stream, killing the 17us cold-start gap.
  - Normalize muls split across vector+scalar so slot-boundary drains don't
    queue behind the mask-add/exp chain; DEPTH=3 consume pipeline.

Per core:
  kT = (x @ W_k)^T, v = x @ W_v, qT = (x_own @ W_q)^T   (bf16 matmuls)
  For each of 4 query slots s (256 queries from super-block [512s, 512s+512)):
    for key block kb in [0, 4s+4): scoresT = kT_blk^T q -> +mask -> exp
      (no max-subtraction: scaled scores are ~N(0,1), exp is fp32-safe)
      denominators via ones-matmul; ctx accumulation in PSUM
    normalize by reciprocal(denom), DMA out (fp32).
"""

import numpy as np

B, S, D = 4, 2048, 1024
NE = D // 128          # contraction chunks (d on partitions)
NKBLK = S // 128       # 128-wide key blocks
NSLOT = 4              # query slots per core
QW = 256               # queries per slot
OWNQ = NSLOT * QW      # 1024 queries per core
MASK_NEG = -1.0e30
SCALE = 1.0 / 32.0     # 1/sqrt(D)
NWARM = 12             # PE warm-up matmuls at kernel start

_cached = {}


def _build():
    import concourse.bacc as bacc
    import concourse.tile as tile
    import concourse.mybir as mybir

    F32 = mybir.dt.float32
    BF16 = mybir.dt.bfloat16
    EXP = mybir.ActivationFunctionType.Exp

    nc = bacc.Bacc("TRN2", target_bir_lowering=False, debug=False, num_devices=8,
                   dynamic_dma_scratch_size=2048)

    xt_d = nc.dram_tensor("xt", [D, S], BF16, kind="ExternalInput")
    xq_d = nc.dram_tensor("xq", [D, OWNQ], BF16, kind="ExternalInput")
    wq_d = nc.dram_tensor("wq", [D, D], BF16, kind="ExternalInput")
    wk_d = nc.dram_tensor("wk", [D, D], BF16, kind="ExternalInput")
    wv_d = nc.dram_tensor("wv", [D, D], BF16, kind="ExternalInput")
    mask_d = nc.dram_tensor("masks", [128, 4 * QW], F32, kind="ExternalInput")
    o_d = nc.dram_tensor("o", [OWNQ, D], F32, kind="ExternalOutput")

    with tile.TileContext(nc) as tc:
        with tc.tile_pool(name="res", bufs=1) as res:
            # column layouts: kT chunk ec at [ec*S + key], v block kb at
            # [kb*D + dout], qT chunk ec at [ec*OWNQ + q]
            kT = res.tile([128, NE * S], BF16, name="kT", tag="kT")
            vv = res.tile([128, NKBLK * D], BF16, name="vv", tag="vv")
            qT = res.tile([128, NE * OWNQ], BF16, name="qT", tag="qT")
            warm_t = res.tile([128, 640], BF16, name="warm_t", tag="warm_t")
            mask_t = res.tile([128, 4 * QW], F32, name="mask_t", tag="mask_t")
            # memset instead of DMA: ready during the entry preamble, ~4us
            # before the first input chunks land, so the PE warm-up (and the
            # HAM un-throttle window) starts as early as possible. Also
            # provides the exact-ones columns the denominator matmuls use.
            nc.gpsimd.memset(warm_t[:, :], 1.0)

            # ---------------- projection phase ----------------
            with (
                tc.tile_pool(name="wpool", bufs=3) as wpool,
                tc.tile_pool(name="xpool", bufs=1) as xpool,
                tc.tile_pool(name="pp", bufs=6, space="PSUM") as pp,
                tc.tile_pool(name="wmp", bufs=2, space="PSUM") as wmp,
            ):
                # Warm-up matmuls on the (tiny, early-arriving) warm tile:
                # keeps PE activity up while the big input DMAs stream, so
                # the HAM clock-gate reaches 2.4GHz before real work starts.
                for i in range(NWARM):
                    wps = wmp.tile([128, 256], F32, name="wps", tag="wps")
                    nc.tensor.matmul(wps[:, :], warm_t[:, 0:128],
                                     warm_t[:, 128:384], start=True, stop=True)

                def load_w(dst, src):
                    # full [D, D] weight as 8 chunk DMAs over the two HWDGE
                    # queues (sync+scalar). The DMA instruction occupies the
                    # issuing engine's FIFO slot for the whole transfer, so
                    # every PSUM->SBUF copy lives on vector (which issues no
                    # DMAs) -- otherwise the copies queue behind the input
                    # stream and the proj pipeline stalls on PSUM slots.
                    for dc in range(NE):
                        eng = nc.sync if dc % 2 == 0 else nc.scalar
                        eng.dma_start(
                            dst[:, dc * D:(dc + 1) * D],
                            src[dc * 128:(dc + 1) * 128, :],
                        )

                wk_t = wpool.tile([128, NE * D], BF16, name="wk_t", tag="w")
                xt_t = xpool.tile([128, NE * S], BF16, name="xt_t", tag="xt")
                # DMA order tracks first use: (wk chunk dc + xt h0 chunk dc)
                # pairs feed the first K groups ASAP, then wv (V starts after
                # one K block), then xt h1 (used from js=2), then wq, xq.
                def load_xt_half(h):
                    for dc in range(NE):
                        eng = nc.sync if dc % 2 == 0 else nc.scalar
                        eng.dma_start(
                            xt_t[:, dc * S + h * 1024: dc * S + (h + 1) * 1024],
                            xt_d[dc * 128:(dc + 1) * 128, h * 1024:(h + 1) * 1024],
                        )

                def load_xt_cols(c0, c1):
                    for dc in range(NE):
                        eng = nc.sync if dc % 2 == 0 else nc.scalar
                        eng.dma_start(
                            xt_t[:, dc * S + c0: dc * S + c1],
                            xt_d[dc * 128:(dc + 1) * 128, c0:c1],
                        )

                # first K group needs wk + xt cols [0:512) only: pair those
                # so the PE ladder starts as early as the HBM stream allows
                for dc in range(NE):
                    eng = nc.sync if dc % 2 == 0 else nc.scalar
                    eng.dma_start(wk_t[:, dc * D:(dc + 1) * D],
                                  wk_d[dc * 128:(dc + 1) * 128, :])
                    eng2 = nc.scalar if dc % 2 == 0 else nc.sync
                    eng2.dma_start(
                        xt_t[:, dc * S: dc * S + 512],
                        xt_d[dc * 128:(dc + 1) * 128, 0:512],
                    )
                wv_t = wpool.tile([128, NE * D], BF16, name="wv_t", tag="w")
                load_w(wv_t, wv_d)
                load_xt_cols(512, 1024)
                load_xt_half(1)
                wq_t = wpool.tile([128, NE * D], BF16, name="wq_t", tag="w")
                load_w(wq_t, wq_d)
                xq_t = xpool.tile([128, NE * OWNQ], BF16, name="xq_t", tag="xq")
                for dc in range(NE):
                    eng = nc.sync if dc % 2 == 0 else nc.scalar
                    eng.dma_start(
                        xq_t[:, dc * OWNQ:(dc + 1) * OWNQ],
                        xq_d[dc * 128:(dc + 1) * 128, :],
                    )
                nc.scalar.dma_start(mask_t[:, :], mask_d[:, :])

                # K pass over the resident x^T; all MMs N=512.
                for js in range(S // 512):
                    for ei in range(NE):
                        ps = pp.tile([128, 512], F32, name="ps_p", tag="ps_p")
                        for dc in range(NE):
                            nc.tensor.matmul(
                                ps[:, :],
                                wk_t[:, dc * D + ei * 128: dc * D + (ei + 1) * 128],
                                xt_t[:, dc * S + js * 512: dc * S + (js + 1) * 512],
                                start=(dc == 0), stop=(dc == NE - 1),
                            )
                        nc.vector.tensor_copy(
                            kT[:, ei * S + js * 512: ei * S + (js + 1) * 512],
                            ps[:, :])

                # Q pass before V so that attention's first score groups
                # (which need kT+qT only) have their inputs long-finished by
                # the time the PE drains the V stream -- no boundary bubble.
                for isl in range(OWNQ // 512):
                    for ei in range(NE):
                        ps = pp.tile([128, 512], F32, name="ps_p", tag="ps_p")
                        for dc in range(NE):
                            nc.tensor.matmul(
                                ps[:, :],
                                wq_t[:, dc * D + ei * 128: dc * D + (ei + 1) * 128],
                                xq_t[:, dc * OWNQ + isl * 512: dc * OWNQ + (isl + 1) * 512],
                                start=(dc == 0), stop=(dc == NE - 1),
                            )
                        nc.vector.tensor_copy(
                            qT[:, ei * OWNQ + isl * 512: ei * OWNQ + (isl + 1) * 512],
                            ps[:, :])

                # V pass
                for kb in range(NKBLK):
                    for dh in range(2):
                        ps = pp.tile([128, 512], F32, name="ps_p", tag="ps_p")
                        for dc in range(NE):
                            nc.tensor.matmul(
                                ps[:, :],
                                xt_t[:, dc * S + kb * 128: dc * S + kb * 128 + 128],
                                wv_t[:, dc * D + dh * 512: dc * D + (dh + 1) * 512],
                                start=(dc == 0), stop=(dc == NE - 1),
                            )
                        nc.vector.tensor_copy(
                            vv[:, kb * D + dh * 512: kb * D + (dh + 1) * 512],
                            ps[:, :],
                        )

            # ---------------- attention phase ----------------
            with (
                tc.tile_pool(name="ptp", bufs=6) as ptp,
                tc.tile_pool(name="obp", bufs=2) as obp,
                tc.tile_pool(name="rcp", bufs=2) as rcp,
                tc.tile_pool(name="scp", bufs=3, space="PSUM") as scp,
                tc.tile_pool(name="ctxp", bufs=1, space="PSUM") as ctxp,
                tc.tile_pool(name="dnp", bufs=1, space="PSUM") as dnp,
            ):
                def consume(item):
                    s, kb, pt, ctx, dn = item
                    nk = 4 * s + 4
                    t_idx = kb - (nk - 4)
                    # Diagonal trim: for diag block t, only queries f >= 64*t
                    # can attend (the rest are fully masked). Scores/exp were
                    # computed only on [q0, 256); dn/ctx must restrict their
                    # stationary (pt) columns the same way — pt[:, :q0] is
                    # uninitialized garbage.
                    q0 = 64 * t_idx if t_idx > 0 else 0
                    for c in range(2):
                        lo = max(q0, c * 128)
                        hi = (c + 1) * 128
                        if lo >= hi:
                            continue  # whole c-half masked for this block
                        # c=0 ends early on trimmed slots (its last two diag
                        # blocks are skipped): close its accumulation group
                        # on its last executed block.
                        last_kb = (nk - 3) if c == 0 else (nk - 1)
                        # Both column groups live in one PSUM bank; start=True
                        # clears the whole bank, so only the first group may
                        # set it — the second lands on freshly cleared psum
                        # (has_written=0) and still overwrites, not adds.
                        nc.tensor.matmul(
                            dn[lo - c * 128: hi - c * 128, 2 * c:2 * c + 2],
                            pt[:, lo:hi],
                            warm_t[:, 0:2],
                            start=(kb == 0 and c == 0), stop=(kb == last_kb),
                            skip_group_check=True,
                        )
                        for dh in range(2):
                            nc.tensor.matmul(
                                ctx[(c, dh)][lo - c * 128: hi - c * 128, :],
                                pt[:, lo:hi],
                                vv[:, kb * D + dh * 512: kb * D + (dh + 1) * 512],
                                start=(kb == 0), stop=(kb == last_kb),
                                skip_group_check=True,
                            )
                    if kb == nk - 1:
                        rc = rcp.tile([128, 2], F32, name="rc", tag="rc")
                        nc.vector.reciprocal(rc[:, :], dn[:, 0:4:2])
                        for c in range(2):
                            ob = obp.tile([128, D], F32, name="ob", tag="ob")
                            for dh in range(2):
                                # split the 4 normalize muls across vector +
                                # scalar so neither queue stalls the next
                                # slot's mask-add/exp chain
                                eng = nc.vector if c == 0 else nc.scalar
                                if eng is nc.vector:
                                    eng.tensor_scalar_mul(
                                        ob[:, dh * 512:(dh + 1) * 512],
                                        ctx[(c, dh)][:, :],
                                        rc[:, c:c + 1],
                                    )
                                else:
                                    eng.mul(
                                        ob[:, dh * 512:(dh + 1) * 512],
                                        ctx[(c, dh)][:, :],
                                        rc[:, c:c + 1],
                                    )
                            eng_d = nc.sync if c == 0 else nc.scalar
                            eng_d.dma_start(
                                o_d[s * QW + c * 128: s * QW + (c + 1) * 128, :],
                                ob[:, :],
                            )

                from collections import deque
                pending = deque()
                DEPTH = 3
                for s in range(NSLOT):
                    nk = 4 * s + 4
                    # Drain before each slot: the slot's ctx/dn pool slots
                    # (bufs=1) can only be re-allocated once the previous
                    # slot's normalize has been emitted.
                    while pending:
                        consume(pending.popleft())
                    ctx_cur = {}
                    for c in range(2):
                        for dh in range(2):
                            t = ctxp.tile(
                                [128, 512], F32,
                                name=f"ctx{c}{dh}", tag=f"ctx{c}{dh}",
                            )
                            ctx_cur[(c, dh)] = t
                    dn_cur = dnp.tile([128, 4], F32, name="dn", tag="dn")
                    for kb in range(nk):
                        t_idx = kb - (nk - 4)
                        q0 = 64 * t_idx if t_idx > 0 else 0
                        qn = QW - q0
                        ps_sc = scp.tile([128, QW], F32, name="ps_sc", tag="sc")
                        for ec in range(NE):
                            nc.tensor.matmul(
                                ps_sc[:, q0:QW],
                                kT[:, ec * S + kb * 128: ec * S + (kb + 1) * 128],
                                qT[:, ec * OWNQ + s * QW + q0: ec * OWNQ + s * QW + QW],
                                start=(ec == 0), stop=(ec == NE - 1),
                            )
                        if t_idx >= 0:
                            nc.vector.tensor_add(
                                ps_sc[:, q0:QW], ps_sc[:, q0:QW],
                                mask_t[:, t_idx * QW + q0:(t_idx + 1) * QW],
                            )
                        pt = ptp.tile([128, QW], BF16, name="pt", tag="pt")
                        nc.scalar.activation(pt[:, q0:QW], ps_sc[:, q0:QW],
                                             EXP, scale=SCALE)
                        pending.append((s, kb, pt, ctx_cur, dn_cur))
                        if len(pending) > DEPTH:
                            consume(pending.popleft())
                while pending:
                    consume(pending.popleft())

    nc.compile()
    return nc


def _get_nc():
    if "nc" not in _cached:
        _cached["nc"] = _build()
    return _cached["nc"]


def build_in_maps(x, W_q, W_k, W_v):
    import ml_dtypes

    BF = ml_dtypes.bfloat16
    x = np.asarray(x, dtype=np.float32)
    wq = np.ascontiguousarray(np.asarray(W_q, dtype=BF))
    wk = np.ascontiguousarray(np.asarray(W_k, dtype=BF))
    wv = np.ascontiguousarray(np.asarray(W_v, dtype=BF))

    p = np.arange(128, dtype=np.int64)[:, None]
    f = np.arange(QW, dtype=np.int64)[None, :]
    masks_h = []
    for h in range(2):
        tiles = [
            np.where(128 * t + p <= 2 * f + h, np.float32(0.0), np.float32(MASK_NEG))
            for t in range(4)
        ]
        masks_h.append(np.concatenate(tiles, axis=1).astype(np.float32))

    xt_b = [np.ascontiguousarray(x[b].T.astype(BF)) for b in range(B)]
    in_maps = []
    for c in range(8):
        b, h = c // 2, c % 2
        xq = np.ascontiguousarray(x[b, h::2, :].T.astype(BF))
        in_maps.append({
            "xt": xt_b[b],
            "xq": xq,
            "wq": wq,
            "wk": wk,
            "wv": wv,
            "masks": masks_h[h],
        })
    return in_maps


def kernel(x, W_q, W_k, W_v):
    from concourse.bass_utils import run_bass_kernel_spmd

    in_maps = build_in_maps(x, W_q, W_k, W_v)
    nc = _get_nc()
    res = run_bass_kernel_spmd(nc, in_maps, core_ids=list(range(8)))

    out = np.empty((B, S, D), dtype=np.float32)
    for c in range(8):
        b, h = c // 2, c % 2
        out[b, h::2, :] = res.results[c]["o"]
    return out
